# revision 1
# baseline (speedup 1.0000x reference)
"""Trainium2 Bass kernel for nn_AdaptiveEmbeddingI2T.

Computes, for image-batch shard i on each of 8 NeuronCores:
  sims[i, b] = <img_vec_i, txt_vec_ib> with
  txt_vec_ib = l2norm_d( mean_t( softmax_t(10*(gam_id*xn_bdt+bet_id)) * (gam*xn+bet) ) )

Key device-side algebra:
  - softmax over t is shift-invariant => exponent = 10*gam*rs*(cap) - 10*gam*rs*mu
    (the +10*bet shift drops; BN normalization folds into a per-channel affine
    applied by the scalar engine's activation scale/bias ports)
  - txt_vec = wscale*S + wbias where S = sum_t(e*cap)/sum_t(e),
    wscale = gam*rs/36, wbias = (bet - gam*rs*mu)/36
  - sims = (sum_d v*w) * rsqrt(sum_d w^2) * rsqrt(sum_d v^2)  (matmuls over d)

Sharding: image batch axis across 8 cores (8 images/core); cap + params
replicated; host concatenates the (8, 64) row blocks.
"""

import sys

if "/opt/trn_rl_repo" not in sys.path:
    sys.path.insert(0, "/opt/trn_rl_repo")

import numpy as np

import concourse.bacc as bacc
import concourse.mybir as mybir
from concourse import masks
from concourse.bass_utils import run_bass_kernel_spmd
from concourse.tile import TileContext

B_IMG, B_CAP, T_CAP, D = 64, 64, 64, 1024
H = 128
T_IMG = 36
EPS = 1e-5
N_CORES = 8
BI = B_IMG // N_CORES          # images per core
R = B_CAP * T_IMG              # 2304 caption rows
NPT = R // 128                 # 18 caption row tiles
NDT = D // 128                 # 8 channel tiles

F32 = mybir.dt.float32
BF16 = mybir.dt.bfloat16

_COMPILED = None


def _build():
    nc = bacc.Bacc("TRN2", target_bir_lowering=False, debug=False,
                   num_devices=N_CORES)
    tensors = _declare_io(nc)
    with TileContext(nc) as tc:
        _emit(nc, tc, *tensors)
    nc.compile()
    return nc


def _emit(nc, tc, cap_d, img_d, wg1_d, wg2_d, wb1_d, wb2_d,
          bg1_d, bb1_d, bg2p1_d, bb2_d, o36_d, ind_d, out_d, reps_main=1):
    AF = mybir.ActivationFunctionType
    ALU = mybir.AluOpType
    AX = mybir.AxisListType
    import os
    _stage = int(os.environ.get("KSTAGE", "7"))
    _wb = int(os.environ.get("KWBUFS", "5"))
    import contextlib
    ctx = contextlib.ExitStack()
    with ctx:
        const = ctx.enter_context(tc.tile_pool(name="const", bufs=1))
        capx = ctx.enter_context(tc.tile_pool(name="capx", bufs=1))
        stream = ctx.enter_context(tc.tile_pool(name="stream", bufs=3))
        imgs = ctx.enter_context(tc.tile_pool(name="imgs", bufs=2))
        work = ctx.enter_context(tc.tile_pool(name="work", bufs=3))
        small = ctx.enter_context(tc.tile_pool(name="small", bufs=1))
        actx = ctx.enter_context(contextlib.ExitStack())
        ppool = actx.enter_context(tc.tile_pool(name="psum", bufs=2, space="PSUM"))
        pacc = actx.enter_context(tc.tile_pool(name="pacc", bufs=1, space="PSUM"))

        ident = const.tile([128, 128], F32)
        masks.make_identity(nc, ident[:])
        _kv = int(os.environ.get("KV", "3"))
        ident_b = const.tile([128, 128], BF16, tag="identb")
        masks.make_identity(nc, ident_b[:])

        # ---- small params ----
        bg1_s = const.tile([H, 1], F32, tag="bg1")
        nc.sync.dma_start(out=bg1_s[:], in_=bg1_d[:])
        bb1_s = const.tile([H, 1], F32, tag="bb1")
        nc.sync.dma_start(out=bb1_s[:], in_=bb1_d[:])
        bg2p1_s = const.tile([128, NDT], F32, tag="bg2p1")
        nc.sync.dma_start(out=bg2p1_s[:], in_=bg2p1_d[:])
        bb2_s = const.tile([128, NDT], F32, tag="bb2t")
        nc.sync.dma_start(out=bb2_s[:], in_=bb2_d[:])
        o36_s = const.tile([T_IMG, 1], F32, tag="o36")
        nc.sync.dma_start(out=o36_s[:], in_=o36_d[:])

        # ---- MLP weights: stream f32 -> resident bf16 ----
        wg1_b = const.tile([128, NDT * H], BF16, tag="wg1b")
        wb1_b = const.tile([128, NDT * H], BF16, tag="wb1b")
        wg2_b = const.tile([128, D], BF16, tag="wg2b")
        wb2_b = const.tile([128, D], BF16, tag="wb2b")
        for w_d, w_b, view in ((wg1_d, wg1_b, True), (wb1_d, wb1_b, True),
                               (wg2_d, wg2_b, False), (wb2_d, wb2_b, False)):
            ws = stream.tile([128, D], F32, tag="stream")
            if view:
                nc.sync.dma_start(
                    out=ws[:].rearrange("p (c h) -> p c h", c=NDT),
                    in_=w_d[:].rearrange("(c p) h -> p c h", p=128))
            else:
                nc.sync.dma_start(out=ws[:], in_=w_d[:])
            nc.vector.tensor_copy(w_b[:], ws[:])

        ones_b = const.tile([128, 1], BF16, tag="onesb")
        nc.gpsimd.memset(ones_b[:], 1.0)

        if _kv == 2:
            ind_b = const.tile([128, NPT * B_CAP], BF16, tag="indb")
            ind_b3 = ind_b[:].rearrange("p (c b) -> p c b", c=NPT)
            wsi = stream.tile([128, NPT * B_CAP], F32, tag="stream")
            nc.sync.dma_start(out=wsi[:], in_=ind_d[:])
            nc.vector.tensor_copy(ind_b[:], wsi[:])

        if _stage < 2:
            res = small.tile([1, BI * B_CAP], F32, tag="res")
            nc.vector.tensor_copy(res[:], bg2p1_s[0:1, :].broadcast_to([1, BI * B_CAP]))
            nc.sync.dma_start(out=out_d[:].rearrange("i b -> (i b)"), in_=res[:])
            return
        # ---- stream cap row-chunks -> transpose -> capTb [d, (b t)] bf16 ----
        capTb = capx.tile([128, NDT * R], BF16, tag="capTb")  # 4.7 MB
        capTb3 = capTb[:].rearrange("p (c r) -> p c r", c=NDT)
        if _kv == 2:
            capRawB = capx.tile([128, NPT * D], BF16, tag="capRawB")  # 4.7 MB
            capRawB3 = capRawB[:].rearrange("p (c d) -> p c d", c=NPT)
        _kcast = os.environ.get("KCAST", "0") == "1"
        for pt in range(NPT):
            if _kcast:
                # f32->bf16 cast during the DMA itself (SWDGE), so the
                # transposes and psum->SBUF copies all run in bf16 (2x)
                crow = stream.tile([128, D], BF16, tag="streamb")
                nc.gpsimd.dma_start(out=crow[:],
                                    in_=cap_d[pt * 128:(pt + 1) * 128, :])
            else:
                crow = stream.tile([128, D], F32, tag="stream")
                nc.sync.dma_start(out=crow[:],
                                  in_=cap_d[pt * 128:(pt + 1) * 128, :])
            if _kv == 2:
                nc.scalar.copy(capRawB3[:, pt, :], crow[:])
            for dh in range(NDT // 2):  # 2 transposes -> one 256-col psum tile
                if _kcast:
                    pt_ps = ppool.tile([128, 256], BF16, tag="trb")
                    idt = ident_b
                else:
                    pt_ps = ppool.tile([128, 256], F32, tag="tr")
                    idt = ident
                for k in range(2):
                    dt = dh * 2 + k
                    nc.tensor.transpose(
                        pt_ps[:, k * 128:(k + 1) * 128],
                        crow[:, dt * 128:(dt + 1) * 128],
                        idt[:])
                for k in range(2):
                    dt = dh * 2 + k
                    dstb = capTb3[:, dt, pt * 128:(pt + 1) * 128]
                    if dh % 2 == 0:
                        nc.vector.tensor_copy(dstb,
                                              pt_ps[:, k * 128:(k + 1) * 128])
                    else:
                        nc.scalar.copy(dstb, pt_ps[:, k * 128:(k + 1) * 128])

        if _stage < 3:
            res = small.tile([1, BI * B_CAP], F32, tag="res")
            nc.vector.tensor_copy(res[:], capTb3[0:1, 0, 0:BI * B_CAP])
            nc.sync.dma_start(out=out_d[:].rearrange("i b -> (i b)"), in_=res[:])
            return
        # ---- BN stats per channel (from bf16 capT) ----
        # sums and sums-of-squares via ACT accum_out (one pass each per dt);
        # variance/rsqrt batched across all dt in a handful of [128,8] ops
        mu = small.tile([128, NDT], F32, tag="mu")
        rs = small.tile([128, NDT], F32, tag="rs")
        musum = small.tile([128, NDT], F32, tag="musum")
        sqsum = small.tile([128, NDT], F32, tag="sqsum")
        tv = small.tile([128, NDT], F32, tag="tv")
        for dt in range(NDT):
            scr = work.tile([128, R], BF16, tag="e", bufs=_wb)
            nc.scalar.activation(scr[:], capTb3[:, dt, :], AF.Copy,
                                 accum_out=musum[:, dt:dt + 1])
            scr2 = work.tile([128, R], BF16, tag="q", bufs=_wb)
            nc.scalar.activation(scr2[:], capTb3[:, dt, :], AF.Square,
                                 accum_out=sqsum[:, dt:dt + 1])
        # var = E[x^2] - mu^2 ; rs = 1/sqrt(var+eps)
        nc.vector.tensor_scalar_mul(mu[:], musum[:], 1.0 / R)
        nc.vector.tensor_tensor(tv[:], mu[:], mu[:], op=ALU.mult)
        nc.vector.tensor_scalar(sqsum[:], sqsum[:], 1.0 / R, None,
                                op0=ALU.mult)
        nc.vector.tensor_tensor(tv[:], sqsum[:], tv[:], op=ALU.subtract)
        nc.vector.tensor_scalar_add(tv[:], tv[:], EPS)
        nc.scalar.sqrt(tv[:], tv[:])
        nc.vector.reciprocal(rs[:], tv[:])

        if _stage < 4:
            res = small.tile([1, BI * B_CAP], F32, tag="res")
            nc.vector.tensor_copy(res[:, 0:NDT], mu[0:1, :])
            nc.sync.dma_start(out=out_d[:].rearrange("i b -> (i b)"), in_=res[:])
            return
        # ---- image means, directly transposed: imgrT [128, (dt i)] ----
        imgrT_ps = pacc.tile([128, NDT * BI], F32, tag="imgrT_ps")
        for i in range(BI):
            ichunk = imgs.tile([T_IMG, D], F32, tag="ichunk")
            nc.sync.dma_start(out=ichunk[:], in_=img_d[i])
            for dt in range(NDT):
                nc.tensor.matmul(
                    imgrT_ps[:, dt * BI + i:dt * BI + i + 1],
                    lhsT=ichunk[:, dt * 128:(dt + 1) * 128], rhs=o36_s[:],
                    start=True, stop=True)

        imgrT = const.tile([128, NDT * BI], F32, tag="imgrT")
        imgrTb = const.tile([128, NDT * BI], BF16, tag="imgrTb")
        imgrT3 = imgrT[:].rearrange("p (c i) -> p c i", c=NDT)
        imgrTb3 = imgrTb[:].rearrange("p (c i) -> p c i", c=NDT)
        nc.vector.tensor_copy(imgrT[:], imgrT_ps[:])
        nc.scalar.copy(imgrTb[:], imgrT_ps[:])

        # rnorm: 1/||v_i|| via accumulating [1,1] matmuls, then transpose
        nrm2_ps = pacc.tile([1, BI], F32, tag="nrm2_ps")
        for i in range(BI):
            for dt in range(NDT):
                nc.tensor.matmul(
                    nrm2_ps[:, i:i + 1],
                    lhsT=imgrT3[:, dt, i:i + 1], rhs=imgrT3[:, dt, i:i + 1],
                    start=(dt == 0), stop=(dt == NDT - 1))
        nrm_row = small.tile([1, BI], F32, tag="nrm_row")
        nc.scalar.sqrt(nrm_row[:], nrm2_ps[:])
        rsr_row = small.tile([1, BI], F32, tag="rsr_row")
        nc.vector.reciprocal(rsr_row[:], nrm_row[:])

        if _stage < 5:
            res = small.tile([1, BI * B_CAP], F32, tag="res")
            nc.vector.tensor_copy(res[:, 0:BI], rsr_row[:])
            nc.sync.dma_start(out=out_d[:].rearrange("i b -> (i b)"), in_=res[:])
            return
        # ---- CBN MLPs -> per-(d,i) scales/biases ----
        wg1_b3 = wg1_b[:].rearrange("p (c h) -> p c h", c=NDT)
        wb1_b3 = wb1_b[:].rearrange("p (c h) -> p c h", c=NDT)

        def mlp_head(w1_b3, b1_s, w2_b, b2_s, name):
            # hT [H, BI] = relu(W1^T imgr^T + b1)
            h_ps = ppool.tile([H, BI], F32, tag="tr")
            for dt in range(NDT):
                nc.tensor.matmul(h_ps[:], lhsT=w1_b3[:, dt, :],
                                 rhs=imgrTb3[:, dt, :],
                                 start=(dt == 0), stop=(dt == NDT - 1))
            hT = small.tile([H, BI], BF16, tag=f"hT_{name}")
            nc.scalar.activation(hT[:], h_ps[:], AF.Relu, bias=b1_s[:], scale=1.0)
            # outT [128, BI] per dt = W2^T h + b2 (+1 folded in b2 for gamma)
            outT = const.tile([128, NDT * BI], F32, tag=f"outT_{name}")
            outT3 = outT[:].rearrange("p (c i) -> p c i", c=NDT)
            for dt in range(NDT):
                o_ps = ppool.tile([128, BI], F32, tag="tr")
                nc.tensor.matmul(o_ps[:], lhsT=w2_b[:, dt * 128:(dt + 1) * 128],
                                 rhs=hT[:], start=True, stop=True)
                nc.scalar.activation(outT3[:, dt, :], o_ps[:], AF.Identity,
                                     bias=b2_s[:, dt:dt + 1], scale=1.0)
            return outT3

        gamT3 = mlp_head(wg1_b3, bg1_s, wg2_b, bg2p1_s, "g")
        betT3 = mlp_head(wb1_b3, bb1_s, wb2_b, bb2_s, "b")

        if _stage < 6:
            res = small.tile([1, BI * B_CAP], F32, tag="res")
            nc.vector.tensor_copy(res[:, 0:BI], gamT3[0:1, 0, :])
            nc.sync.dma_start(out=out_d[:].rearrange("i b -> (i b)"), in_=res[:])
            return
        # escale = 10*gam*rs ; ebias = -escale*mu
        # wscale = gam*rs/36 ; wbias = (bet - gam*rs*mu)/36
        escale = const.tile([128, NDT * BI], F32, tag="escale")
        ebias = const.tile([128, NDT * BI], F32, tag="ebias")
        wscale = const.tile([128, NDT * BI], F32, tag="wscale")
        wbias = const.tile([128, NDT * BI], F32, tag="wbias")
        es3 = escale[:].rearrange("p (c i) -> p c i", c=NDT)
        eb3 = ebias[:].rearrange("p (c i) -> p c i", c=NDT)
        ws3 = wscale[:].rearrange("p (c i) -> p c i", c=NDT)
        wb3 = wbias[:].rearrange("p (c i) -> p c i", c=NDT)
        grs = small.tile([128, BI], F32, tag="grs")
        tmp = small.tile([128, BI], F32, tag="tmpb")
        negmu = small.tile([128, 1], F32, tag="negmu")
        for dt in range(NDT):
            nc.vector.tensor_scalar(grs[:], gamT3[:, dt, :], rs[:, dt:dt + 1],
                                    None, op0=ALU.mult)
            nc.vector.tensor_scalar_mul(es3[:, dt, :], grs[:], 10.0)
            nc.vector.tensor_scalar_mul(negmu[:], mu[:, dt:dt + 1], -1.0)
            nc.vector.tensor_scalar(eb3[:, dt, :], es3[:, dt, :], negmu[:],
                                    None, op0=ALU.mult)
            nc.vector.tensor_scalar_mul(ws3[:, dt, :], grs[:], 1.0 / 36.0)
            nc.vector.tensor_scalar(tmp[:], grs[:], mu[:, dt:dt + 1],
                                    None, op0=ALU.mult)
            nc.vector.tensor_tensor(tmp[:], betT3[:, dt, :], tmp[:],
                                    op=ALU.subtract)
            nc.vector.tensor_scalar_mul(wb3[:, dt, :], tmp[:], 1.0 / 36.0)

        if _stage < 7:
            res = small.tile([1, BI * B_CAP], F32, tag="res")
            nc.vector.tensor_copy(res[:, 0:BI], es3[0:1, 0, :])
            nc.sync.dma_start(out=out_d[:].rearrange("i b -> (i b)"), in_=res[:])
            return
        # ---- main loop over (i, dt) ----
        _ilim = int(os.environ.get("KILIM", str(BI)))
        _dtlim = int(os.environ.get("KDTLIM", str(NDT)))
        _kv = int(os.environ.get("KV", "3"))
        actx.close()  # release phase-A psum banks
        pmain = ctx.enter_context(tc.tile_pool(name="pmain", bufs=1,
                                               space="PSUM"))
        dot_ps = pmain.tile([1, BI * B_CAP], F32, tag="dot_ps")
        nrm_ps = pmain.tile([1, BI * B_CAP], F32, tag="nrm_ps")
        NH = NPT // 2
        _kb = os.environ.get("KB", "1") == "1"
        for _rep in range(reps_main):
            for i in range(_ilim):
                if _kv == 3 and _kb:
                    # per-i batched: trees fill s1_all/s2_all, then one recip+mult
                    s12_all = work.tile([128, 2 * NDT * B_CAP], F32,
                                        tag="s12all")
                    s12v = s12_all[:].rearrange("p (s c b) -> p s c b", s=2,
                                                c=NDT)
                    _kf = os.environ.get("KFUSE", "0") == "1"
                    for dt in range(_dtlim):
                        if not _kf:
                            e_t = work.tile([128, R], BF16, tag="e",
                                            bufs=_wb)
                            nc.scalar.activation(e_t[:], capTb3[:, dt, :],
                                                 AF.Exp,
                                                 bias=eb3[:, dt, i:i + 1],
                                                 scale=es3[:, dt, i:i + 1])
                            q_t = work.tile([128, R], BF16, tag="q",
                                            bufs=_wb)
                            nc.vector.tensor_tensor(q_t[:], e_t[:],
                                                    capTb3[:, dt, :],
                                                    op=ALU.mult)

                            def tsum(srct, dst, name):
                                v = srct[:].rearrange("p (b t) -> p b t",
                                                      t=T_IMG)
                                a1_ = work.tile([128, B_CAP * 18], BF16,
                                                tag=f"{name}a1")
                                a13 = a1_[:].rearrange("p (b t) -> p b t",
                                                       t=18)
                                nc.vector.tensor_tensor(a13, v[:, :, 0:18],
                                                        v[:, :, 18:36],
                                                        op=ALU.add)
                                a2_ = work.tile([128, B_CAP * 9], BF16,
                                                tag=f"{name}a2")
                                a23 = a2_[:].rearrange("p (b t) -> p b t",
                                                       t=9)
                                nc.vector.tensor_tensor(a23, a13[:, :, 0:9],
                                                        a13[:, :, 9:18],
                                                        op=ALU.add)
                                nc.vector.tensor_reduce(dst, a23, axis=AX.X,
                                                        op=ALU.add)

                            tsum(e_t, s12v[:, 0, dt, :], "s1")
                            tsum(q_t, s12v[:, 1, dt, :], "s2")
                            continue
                        # e and q adjacent in one tile so both t-sum trees run
                        # as single wide DVE instructions (fewer op overheads)
                        eq = work.tile([128, 2 * R], BF16, tag="eq")
                        e_t = eq[:, 0:R]
                        q_t = eq[:, R:2 * R]
                        nc.scalar.activation(e_t, capTb3[:, dt, :], AF.Exp,
                                             bias=eb3[:, dt, i:i + 1],
                                             scale=es3[:, dt, i:i + 1])
                        nc.vector.tensor_tensor(q_t, e_t, capTb3[:, dt, :],
                                                op=ALU.mult)
                        eqv = eq[:].rearrange("p (s b t) -> p s b t", s=2,
                                              t=T_IMG)
                        a1 = work.tile([128, 2 * B_CAP * 18], BF16,
                                       tag="eqa1")
                        a1v = a1[:].rearrange("p (s b t) -> p s b t", s=2,
                                              t=18)
                        nc.vector.tensor_tensor(a1v, eqv[:, :, :, 0:18],
                                                eqv[:, :, :, 18:36],
                                                op=ALU.add)
                        a2 = work.tile([128, 2 * B_CAP * 9], BF16,
                                       tag="eqa2")
                        a2v = a2[:].rearrange("p (s b t) -> p s b t", s=2,
                                              t=9)
                        nc.vector.tensor_tensor(a2v, a1v[:, :, :, 0:9],
                                                a1v[:, :, :, 9:18],
                                                op=ALU.add)
                        nc.vector.tensor_reduce(
                            s12v[:, :, dt, :], a2v, axis=AX.X, op=ALU.add)
                    r1_all = work.tile([128, NDT * B_CAP], F32, tag="r1all")
                    nc.vector.reciprocal(r1_all[:],
                                         s12_all[:, 0:NDT * B_CAP])
                    sc_all = work.tile([128, NDT * B_CAP], F32, tag="scall")
                    nc.vector.tensor_tensor(
                        sc_all[:], s12_all[:, NDT * B_CAP:2 * NDT * B_CAP],
                        r1_all[:], op=ALU.mult)
                    for dt in range(_dtlim):
                        w_t = work.tile([128, B_CAP], BF16, tag="w")
                        nc.scalar.activation(
                            w_t[:], sc_all[:, dt * B_CAP:(dt + 1) * B_CAP],
                            AF.Identity, bias=wb3[:, dt, i:i + 1],
                            scale=ws3[:, dt, i:i + 1])
                        w2_t = work.tile([128, B_CAP], BF16, tag="w2")
                        nc.scalar.square(w2_t[:], w_t[:])
                        nc.tensor.matmul(dot_ps[:, i * B_CAP:(i + 1) * B_CAP],
                                         lhsT=imgrTb3[:, dt, i:i + 1], rhs=w_t[:],
                                         start=(dt == 0), stop=(dt == _dtlim - 1))
                        nc.tensor.matmul(nrm_ps[:, i * B_CAP:(i + 1) * B_CAP],
                                         lhsT=ones_b[:], rhs=w2_t[:],
                                         start=(dt == 0), stop=(dt == _dtlim - 1))
                    continue
                for dt in range(_dtlim):

                    e_t = work.tile([128, R], BF16, tag="e")
                    nc.scalar.activation(e_t[:], capTb3[:, dt, :], AF.Exp,
                                         bias=eb3[:, dt, i:i + 1],
                                         scale=es3[:, dt, i:i + 1])
                    if _kv == 2:
                        # transpose e -> [(b t) rows, d] chunks; s1/s2 via
                        # indicator matmuls on PE (data-as-weights)
                        eT_sb = work.tile([128, NPT * 128], BF16, tag="eT")
                        for h in range(2):
                            eT_ps = pmain.tile([128, NH * 128], BF16,
                                               tag=f"eTps{h}")
                            for c9 in range(NH):
                                c = h * NH + c9
                                nc.tensor.transpose(
                                    eT_ps[:, c9 * 128:(c9 + 1) * 128],
                                    e_t[:, c * 128:(c + 1) * 128], ident_b[:])
                            nc.vector.tensor_copy(
                                eT_sb[:, h * NH * 128:(h + 1) * NH * 128],
                                eT_ps[:])
                        eT3 = eT_sb[:].rearrange("p (c d) -> p c d", c=NPT)
                        qT = work.tile([128, NPT * 128], BF16, tag="qT")
                        qT3 = qT[:].rearrange("p (c d) -> p c d", c=NPT)
                        nc.vector.tensor_tensor(
                            qT3, eT3, capRawB3[:, :, dt * 128:(dt + 1) * 128],
                            op=ALU.mult)
                        s1_ps = pmain.tile([128, B_CAP], F32, tag="s1ps")
                        s2_ps = pmain.tile([128, B_CAP], F32, tag="s2ps")
                        for c in range(NPT):
                            nc.tensor.matmul(s1_ps[:], lhsT=eT3[:, c, :],
                                             rhs=ind_b3[:, c, :],
                                             start=(c == 0), stop=(c == NPT - 1))
                        for c in range(NPT):
                            nc.tensor.matmul(s2_ps[:], lhsT=qT3[:, c, :],
                                             rhs=ind_b3[:, c, :],
                                             start=(c == 0), stop=(c == NPT - 1))
                        r1 = work.tile([128, B_CAP], F32, tag="r1")
                        nc.vector.reciprocal(r1[:], s1_ps[:])
                        sc = work.tile([128, B_CAP], F32, tag="sc")
                        nc.vector.tensor_tensor(sc[:], s2_ps[:], r1[:],
                                                op=ALU.mult)
                    elif _kv == 3:
                        # q split GPSIMD/DVE; t-sums via bf16 tree adds (2x mode)
                        _qs = int(os.environ.get("KQSPLIT", "0"))
                        q_t = work.tile([128, R], BF16, tag="q")
                        if _qs > 0:
                            nc.gpsimd.tensor_tensor(
                                q_t[:, 0:_qs], e_t[:, 0:_qs],
                                capTb3[:, dt, 0:_qs], op=ALU.mult)
                        if _qs < R:
                            nc.vector.tensor_tensor(
                                q_t[:, _qs:R], e_t[:, _qs:R],
                                capTb3[:, dt, _qs:R], op=ALU.mult)

                        def tsum(src, name):
                            v = src[:].rearrange("p (b t) -> p b t", t=T_IMG)
                            a1 = work.tile([128, B_CAP * 18], BF16,
                                           tag=f"{name}a1")
                            a13 = a1[:].rearrange("p (b t) -> p b t", t=18)
                            nc.vector.tensor_tensor(a13, v[:, :, 0:18],
                                                    v[:, :, 18:36], op=ALU.add)
                            a2 = work.tile([128, B_CAP * 9], BF16,
                                           tag=f"{name}a2")
                            a23 = a2[:].rearrange("p (b t) -> p b t", t=9)
                            nc.vector.tensor_tensor(a23, a13[:, :, 0:9],
                                                    a13[:, :, 9:18], op=ALU.add)
                            s_f = work.tile([128, B_CAP], F32, tag=f"{name}s")
                            nc.vector.tensor_reduce(s_f[:], a23, axis=AX.X,
                                                    op=ALU.add)
                            return s_f

                        s1 = tsum(e_t, "s1")
                        s2 = tsum(q_t, "s2")
                        r1 = work.tile([128, B_CAP], F32, tag="r1")
                        nc.vector.reciprocal(r1[:], s1[:])
                        sc = work.tile([128, B_CAP], F32, tag="sc")
                        nc.vector.tensor_tensor(sc[:], s2[:], r1[:], op=ALU.mult)
                    else:
                        q_t = work.tile([128, R], BF16, tag="q")
                        nc.vector.tensor_tensor(q_t[:], e_t[:], capTb3[:, dt, :],
                                                op=ALU.mult)
                        s1 = work.tile([128, B_CAP], F32, tag="s1")
                        nc.vector.tensor_reduce(
                            s1[:], e_t[:].rearrange("p (b t) -> p b t", t=T_IMG),
                            axis=AX.X, op=ALU.add)
                        s2 = work.tile([128, B_CAP], F32, tag="s2")
                        nc.vector.tensor_reduce(
                            s2[:], q_t[:].rearrange("p (b t) -> p b t", t=T_IMG),
                            axis=AX.X, op=ALU.add)
                        r1 = work.tile([128, B_CAP], F32, tag="r1")
                        nc.vector.reciprocal(r1[:], s1[:])
                        sc = work.tile([128, B_CAP], F32, tag="sc")
                        nc.vector.tensor_tensor(sc[:], s2[:], r1[:], op=ALU.mult)
                    w_t = work.tile([128, B_CAP], BF16, tag="w")
                    nc.scalar.activation(w_t[:], sc[:], AF.Identity,
                                         bias=wb3[:, dt, i:i + 1],
                                         scale=ws3[:, dt, i:i + 1])
                    w2_t = work.tile([128, B_CAP], BF16, tag="w2")
                    nc.scalar.square(w2_t[:], w_t[:])
                    nc.tensor.matmul(dot_ps[:, i * B_CAP:(i + 1) * B_CAP],
                                     lhsT=imgrTb3[:, dt, i:i + 1], rhs=w_t[:],
                                     start=(dt == 0), stop=(dt == _dtlim - 1))
                    nc.tensor.matmul(nrm_ps[:, i * B_CAP:(i + 1) * B_CAP],
                                     lhsT=ones_b[:], rhs=w2_t[:],
                                     start=(dt == 0), stop=(dt == _dtlim - 1))

            # ---- epilogue: sims = dot * rsqrt(nrm) * (1/|v|) ----
            rr = small.tile([1, BI * B_CAP], F32, tag="rr")
            nc.vector.reciprocal(rr[:], nrm_ps[:])
            rsn = small.tile([1, BI * B_CAP], F32, tag="rsn")
            nc.scalar.sqrt(rsn[:], rr[:])
            prod = small.tile([1, BI * B_CAP], F32, tag="prod")
            nc.vector.tensor_tensor(prod[:], dot_ps[:], rsn[:], op=ALU.mult)
            res = small.tile([1, BI * B_CAP], F32, tag="res")
            rsr_b = rsr_row[:].rearrange("p (i u) -> p i u", u=1).broadcast_to([1, BI, B_CAP])
            nc.vector.tensor_tensor(
                res[:].rearrange("p (i b) -> p i b", i=BI),
                prod[:].rearrange("p (i b) -> p i b", i=BI),
                rsr_b, op=ALU.mult)
            nc.sync.dma_start(out=out_d[:].rearrange("i b -> (i b)"), in_=res[:])


def _get_compiled():
    global _COMPILED
    if _COMPILED is None:
        _COMPILED = _build()
    return _COMPILED


def _declare_io(nc):
    return (
        nc.dram_tensor("cap", [R, D], F32, kind="ExternalInput"),
        nc.dram_tensor("img", [BI, T_IMG, D], F32, kind="ExternalInput"),
        nc.dram_tensor("wg1", [D, H], F32, kind="ExternalInput"),
        nc.dram_tensor("wg2", [H, D], F32, kind="ExternalInput"),
        nc.dram_tensor("wb1", [D, H], F32, kind="ExternalInput"),
        nc.dram_tensor("wb2", [H, D], F32, kind="ExternalInput"),
        nc.dram_tensor("bg1", [H, 1], F32, kind="ExternalInput"),
        nc.dram_tensor("bb1", [H, 1], F32, kind="ExternalInput"),
        nc.dram_tensor("bg2p1", [128, NDT], F32, kind="ExternalInput"),
        nc.dram_tensor("bb2t", [128, NDT], F32, kind="ExternalInput"),
        nc.dram_tensor("o36", [T_IMG, 1], F32, kind="ExternalInput"),
        nc.dram_tensor("ind", [128, NPT * B_CAP], F32, kind="ExternalInput"),
        nc.dram_tensor("out", [BI, B_CAP], F32, kind="ExternalOutput"),
    )


def _build_repeated(reps):
    """Timing variant: run the compute `reps` times in one NEFF. With
    KREPMODE=main, phase A runs once and only the main loop repeats."""
    import os
    nc = bacc.Bacc("TRN2", target_bir_lowering=False, debug=False,
                   num_devices=N_CORES)
    tensors = _declare_io(nc)
    with TileContext(nc) as tc:
        if os.environ.get("KREPMODE") == "main":
            _emit(nc, tc, *tensors, reps_main=reps)
        else:
            for _ in range(reps):
                _emit(nc, tc, *tensors)
    nc.compile()
    return nc


def _indicator():
    ind = np.zeros((128, NPT, B_CAP), np.float32)
    for c in range(NPT):
        for r in range(128):
            ind[r, c, (c * 128 + r) // T_IMG] = 1.0
    return ind.reshape(128, NPT * B_CAP)


def _in_maps(img_embed, cap_embed, Wg1, bg1, Wg2, bg2, Wb1, bb1, Wb2, bb2):
    cap = np.ascontiguousarray(
        cap_embed[:, :T_IMG, :].reshape(R, D)).astype(np.float32)
    shared = {
        "cap": cap,
        "wg1": np.ascontiguousarray(Wg1, np.float32),
        "wg2": np.ascontiguousarray(Wg2, np.float32),
        "wb1": np.ascontiguousarray(Wb1, np.float32),
        "wb2": np.ascontiguousarray(Wb2, np.float32),
        "bg1": np.ascontiguousarray(bg1.reshape(H, 1), np.float32),
        "bb1": np.ascontiguousarray(bb1.reshape(H, 1), np.float32),
        "bg2p1": np.ascontiguousarray((bg2 + 1.0).reshape(NDT, 128).T,
                                      np.float32),
        "bb2t": np.ascontiguousarray(bb2.reshape(NDT, 128).T, np.float32),
        "o36": np.full((T_IMG, 1), 1.0 / T_IMG, np.float32),
        "ind": _indicator(),
    }
    maps = []
    for c in range(N_CORES):
        m = dict(shared)
        m["img"] = np.ascontiguousarray(
            img_embed[c * BI:(c + 1) * BI], np.float32)
        maps.append(m)
    return maps


def kernel(img_embed, cap_embed, lens, Wg1, bg1, Wg2, bg2, Wb1, bb1, Wb2, bb2):
    del lens  # unused by the reference computation
    nc = _get_compiled()
    maps = _in_maps(np.asarray(img_embed), np.asarray(cap_embed),
                    np.asarray(Wg1), np.asarray(bg1), np.asarray(Wg2),
                    np.asarray(bg2), np.asarray(Wb1), np.asarray(bb1),
                    np.asarray(Wb2), np.asarray(bb2))
    import time as _time
    last = None
    for attempt in range(5):  # device occasionally needs runs to recover
        try:
            res = run_bass_kernel_spmd(nc, maps, core_ids=list(range(N_CORES)))
            break
        except Exception as e:
            last = e
            _time.sleep(10)
    else:
        raise last
    return np.concatenate([res.results[c]["out"] for c in range(N_CORES)],
                          axis=0).astype(np.float32)



# revision 33
# speedup vs baseline: 1.1206x; 1.1206x over previous
"""Trainium2 Bass kernel for nn_AdaptiveEmbeddingI2T.

Computes, for image-batch shard i on each of 8 NeuronCores:
  sims[i, b] = <img_vec_i, txt_vec_ib> with
  txt_vec_ib = l2norm_d( mean_t( softmax_t(10*(gam_id*xn_bdt+bet_id)) * (gam*xn+bet) ) )

Device-side algebra (per image i, channel d, caption b, time t):
  - softmax over t is shift/scale invariant in the ratio
      sc[d,b] = sum_t(e*cap) / sum_t(e),  e = exp(es[d,i]*cap[d,b,t]),
      es = 10*gam*rs   (the -es*mu shift and exp(bias) factor cancel)
  - txt_vec ~ w' = es*sc + (10*bet - es*mu)   (any uniform scale of w'
      cancels in the final l2 normalization, so the /36 and /10 drop)
  - sims = (sum_d v*w') * rsqrt(sum_d w'^2) * rsqrt(sum_d v^2)

Engine mapping:
  - exp on ACT in [d-partition, (b t)] layout (per-partition scale port)
  - e -> eT: one DMA-XBAR transpose for the first KXJ row-chunks, PE
    transposes + DVE psum->sbuf copy for the rest (balances DMA vs PE/DVE)
  - qT = eT * capR elementwise on DVE (the big DVE op)
  - s1 = sum_t e and s2 = sum_t q as PE indicator matmuls over the
    r=(b,t) partition chunks, accumulating f32 in PSUM (no DVE trees)
  - w' affine on DVE (tensor_scalar), w'^2 on GPSIMD, dots on PE
  - BN stats via PE matmuls (ones / self) off capR, diag extract on DVE
  - all rsqrt via Exp(-0.5*Ln(x)) so every ACT func lives in the single
    natural_log_exp_and_others table (no act-table reloads)

Sharding: image batch axis across 8 cores (8 images/core); cap + params
replicated; host concatenates the (8, 64) row blocks.
"""

import os
import sys

if "/opt/trn_rl_repo" not in sys.path:
    sys.path.insert(0, "/opt/trn_rl_repo")

import numpy as np
import ml_dtypes

import concourse.bacc as bacc
import concourse.mybir as mybir
from concourse import masks
from concourse.bass_utils import run_bass_kernel_spmd
from concourse.tile import TileContext

B_IMG, B_CAP, T_CAP, D = 64, 64, 64, 1024
H = 128
T_IMG = 36
EPS = 1e-5
N_CORES = 8
BI = B_IMG // N_CORES          # images per core
R = B_CAP * T_IMG              # 2304 caption rows
NPT = R // 128                 # 18 caption row chunks
NDT = D // 128                 # 8 channel tiles

F32 = mybir.dt.float32
BF16 = mybir.dt.bfloat16
BF16_NP = ml_dtypes.bfloat16

AF = mybir.ActivationFunctionType
ALU = mybir.AluOpType
AX = mybir.AxisListType

_COMPILED = None


def _declare_io(nc):
    return (
        nc.dram_tensor("capt", [D, R], BF16, kind="ExternalInput"),
        nc.dram_tensor("capr", [R, D], BF16, kind="ExternalInput"),
        nc.dram_tensor("ind", [128, NPT * B_CAP], BF16, kind="ExternalInput"),
        nc.dram_tensor("img", [BI, T_IMG, D], F32, kind="ExternalInput"),
        nc.dram_tensor("wg1b", [128, NDT * H], BF16, kind="ExternalInput"),
        nc.dram_tensor("wg2b", [H, D], BF16, kind="ExternalInput"),
        nc.dram_tensor("wb1b", [128, NDT * H], BF16, kind="ExternalInput"),
        nc.dram_tensor("wb2b", [H, D], BF16, kind="ExternalInput"),
        nc.dram_tensor("bg1", [H, 1], F32, kind="ExternalInput"),
        nc.dram_tensor("bb1", [H, 1], F32, kind="ExternalInput"),
        nc.dram_tensor("bg2p1", [128, NDT], F32, kind="ExternalInput"),
        nc.dram_tensor("bb2t", [128, NDT], F32, kind="ExternalInput"),
        nc.dram_tensor("o36", [T_IMG, 1], F32, kind="ExternalInput"),
        nc.dram_tensor("out", [BI, B_CAP], F32, kind="ExternalOutput"),
    )


def _emit(nc, tc, capt_d, capr_d, ind_d, img_d, wg1_d, wg2_d, wb1_d, wb2_d,
          bg1_d, bb1_d, bg2p1_d, bb2_d, o36_d, out_d, reps_main=1):
    import contextlib
    ctx = contextlib.ExitStack()
    _xj = int(os.environ.get("KXJ", "12"))        # chunks via XBAR
    with ctx:
        const = ctx.enter_context(tc.tile_pool(name="const", bufs=2))
        capx = ctx.enter_context(tc.tile_pool(name="capx", bufs=1))
        imgs = ctx.enter_context(tc.tile_pool(name="imgs", bufs=2))
        work = ctx.enter_context(tc.tile_pool(name="work", bufs=3))
        small = ctx.enter_context(tc.tile_pool(name="small", bufs=1))
        actx = ctx.enter_context(contextlib.ExitStack())
        ppool = actx.enter_context(tc.tile_pool(name="psum", bufs=1,
                                                space="PSUM"))
        pacc = actx.enter_context(tc.tile_pool(name="pacc", bufs=1,
                                               space="PSUM"))

        ident = const.tile([128, 128], F32, bufs=1)
        masks.make_identity(nc, ident[:])
        ident_b = const.tile([128, 128], BF16, tag="identb", bufs=1)
        masks.make_identity(nc, ident_b[:])
        ones_b = const.tile([128, 1], BF16, tag="onesb", bufs=1)
        nc.gpsimd.memset(ones_b[:], 1.0)

        # ---- loads: smalls/weights/img first, then capR (stats), capT ----
        bg1_s = const.tile([H, 1], F32, tag="bg1", bufs=1)
        nc.sync.dma_start(out=bg1_s[:], in_=bg1_d[:])
        bb1_s = const.tile([H, 1], F32, tag="bb1", bufs=1)
        nc.sync.dma_start(out=bb1_s[:], in_=bb1_d[:])
        bg2p1_s = const.tile([128, NDT], F32, tag="bg2p1", bufs=1)
        nc.sync.dma_start(out=bg2p1_s[:], in_=bg2p1_d[:])
        bb2_s = const.tile([128, NDT], F32, tag="bb2t", bufs=1)
        nc.sync.dma_start(out=bb2_s[:], in_=bb2_d[:])
        o36_s = const.tile([T_IMG, 1], F32, tag="o36", bufs=1)
        nc.sync.dma_start(out=o36_s[:], in_=o36_d[:])
        wg1_b = const.tile([128, NDT * H], BF16, tag="wg1b", bufs=1)
        nc.sync.dma_start(out=wg1_b[:], in_=wg1_d[:])
        wb1_b = const.tile([128, NDT * H], BF16, tag="wb1b", bufs=1)
        nc.sync.dma_start(out=wb1_b[:], in_=wb1_d[:])
        wg2_b = const.tile([128, D], BF16, tag="wg2b", bufs=1)
        nc.sync.dma_start(out=wg2_b[:], in_=wg2_d[:])
        wb2_b = const.tile([128, D], BF16, tag="wb2b", bufs=1)
        nc.sync.dma_start(out=wb2_b[:], in_=wb2_d[:])
        img_tiles = []
        for i in range(BI):
            ichunk = imgs.tile([T_IMG, D], F32, tag="ichunk", bufs=4)
            nc.sync.dma_start(out=ichunk[:], in_=img_d[i])
            img_tiles.append(ichunk)

        ind_s = capx.tile([128, NPT * B_CAP], BF16, tag="ind", bufs=2)
        ind3 = ind_s[:].rearrange("p (c b) -> p c b", c=NPT)
        nc.sync.dma_start(out=ind_s[:], in_=ind_d[:])
        capR = capx.tile([128, NPT * D], BF16, tag="capR", bufs=2)
        capR3 = capR[:].rearrange("p (c d) -> p c d", c=NPT)
        capr_v = capr_d[:].rearrange("(c p) d -> p c d", p=128)
        for c in range(NPT):
            nc.sync.dma_start(out=capR3[:, c, :], in_=capr_v[:, c, :])
        capT = capx.tile([128, NDT * R], BF16, tag="capT", bufs=1)
        capT3 = capT[:].rearrange("p (c r) -> p c r", c=NDT)
        capt_v = capt_d[:].rearrange("(c p) r -> p c r", p=128)
        for dt in range(NDT):
            nc.sync.dma_start(out=capT3[:, dt, :], in_=capt_v[:, dt, :])

        # ---- BN stats on PE off capR chunks (chunk-major) ----
        mus_ps = pacc.tile([128, NDT], F32, tag="mus_ps")
        for c in range(NPT):
            for dt in range(NDT):
                nc.tensor.matmul(mus_ps[:, dt:dt + 1],
                                 lhsT=capR3[:, c, dt * 128:(dt + 1) * 128],
                                 rhs=ones_b[:],
                                 start=(c == 0), stop=(c == NPT - 1))
        sqsum = small.tile([128, NDT], F32, tag="sqsum", bufs=2)
        sq_tiles = []
        for k in range(NDT):
            sq_t = ppool.tile([128, 128], F32, tag=f"sq_ps{k % 3}")
            sq_tiles.append(sq_t)
        for wave, dts in enumerate((range(0, 3), range(3, 6), range(6, 8))):
            for c in range(NPT):
                for dt in dts:
                    nc.tensor.matmul(
                        sq_tiles[dt][:],
                        lhsT=capR3[:, c, dt * 128:(dt + 1) * 128],
                        rhs=capR3[:, c, dt * 128:(dt + 1) * 128],
                        start=(c == 0), stop=(c == NPT - 1))
            for dt in dts:
                dg = work.tile([128, 128], F32, tag="dg", bufs=2)
                nc.vector.tensor_tensor(dg[:], sq_tiles[dt][:], ident[:],
                                        op=ALU.mult)
                nc.vector.tensor_reduce(
                    sqsum[:, dt:dt + 1],
                    dg[:].rearrange("p (u q) -> p u q", u=1),
                    axis=AX.X, op=ALU.add)

        # mu = musum/R ; var = E[x^2]-mu^2 ; rs = Exp(-0.5*Ln(var+eps))
        mu = small.tile([128, NDT], F32, tag="mu", bufs=2)
        rs = small.tile([128, NDT], F32, tag="rs", bufs=2)
        tv = small.tile([128, NDT], F32, tag="tv", bufs=2)
        nc.vector.tensor_scalar_mul(mu[:], mus_ps[:], 1.0 / R)
        nc.vector.tensor_tensor(tv[:], mu[:], mu[:], op=ALU.mult)
        nc.vector.tensor_scalar(sqsum[:], sqsum[:], 1.0 / R, None,
                                op0=ALU.mult)
        nc.vector.tensor_tensor(tv[:], sqsum[:], tv[:], op=ALU.subtract)
        nc.vector.tensor_scalar_add(tv[:], tv[:], EPS)
        nc.scalar.activation(tv[:], tv[:], AF.Ln)
        nc.scalar.activation(rs[:], tv[:], AF.Exp, scale=-0.5)

        # ---- image means, directly transposed: imgrT [128, (dt i)] ----
        imgrT_ps = pacc.tile([128, NDT * BI], F32, tag="imgrT_ps")
        for i in range(BI):
            ichunk = img_tiles[i]
            for dt in range(NDT):
                nc.tensor.matmul(
                    imgrT_ps[:, dt * BI + i:dt * BI + i + 1],
                    lhsT=ichunk[:, dt * 128:(dt + 1) * 128], rhs=o36_s[:],
                    start=True, stop=True)

        imgrT = const.tile([128, NDT * BI], F32, tag="imgrT")
        imgrTb = const.tile([128, NDT * BI], BF16, tag="imgrTb")
        imgrT3 = imgrT[:].rearrange("p (c i) -> p c i", c=NDT)
        imgrTb3 = imgrTb[:].rearrange("p (c i) -> p c i", c=NDT)
        nc.vector.tensor_copy(imgrT[:], imgrT_ps[:])
        nc.scalar.copy(imgrTb[:], imgrT_ps[:])

        # 1/||v_i|| via accumulating [1,1] matmuls, rsqrt via Ln/Exp
        nrm2_ps = pacc.tile([1, BI], F32, tag="nrm2_ps")
        for i in range(BI):
            for dt in range(NDT):
                nc.tensor.matmul(
                    nrm2_ps[:, i:i + 1],
                    lhsT=imgrT3[:, dt, i:i + 1], rhs=imgrT3[:, dt, i:i + 1],
                    start=(dt == 0), stop=(dt == NDT - 1))
        rsr_row = small.tile([1, BI], F32, tag="rsr_row", bufs=2)
        nc.scalar.activation(rsr_row[:], nrm2_ps[:], AF.Ln)
        nc.scalar.activation(rsr_row[:], rsr_row[:], AF.Exp, scale=-0.5)

        # ---- CBN MLPs -> gamT/betT [128, (dt, i)] f32 ----
        wg1_b3 = wg1_b[:].rearrange("p (c h) -> p c h", c=NDT)
        wb1_b3 = wb1_b[:].rearrange("p (c h) -> p c h", c=NDT)

        def mlp_head(w1_b3, b1_s, w2_b, b2_s, name):
            h_ps = ppool.tile([H, BI], F32, tag="h_ps")
            for dt in range(NDT):
                nc.tensor.matmul(h_ps[:], lhsT=w1_b3[:, dt, :],
                                 rhs=imgrTb3[:, dt, :],
                                 start=(dt == 0), stop=(dt == NDT - 1))
            hT = small.tile([H, BI], BF16, tag=f"hT_{name}", bufs=2)
            nc.scalar.activation(hT[:], h_ps[:], AF.Relu, bias=b1_s[:],
                                 scale=1.0)
            outT = const.tile([128, NDT * BI], F32, tag=f"outT_{name}")
            outT3 = outT[:].rearrange("p (c i) -> p c i", c=NDT)
            for dt in range(NDT):
                o_ps = ppool.tile([128, BI], F32, tag="o_ps")
                nc.tensor.matmul(o_ps[:],
                                 lhsT=w2_b[:, dt * 128:(dt + 1) * 128],
                                 rhs=hT[:], start=True, stop=True)
                nc.scalar.activation(outT3[:, dt, :], o_ps[:], AF.Identity,
                                     bias=b2_s[:, dt:dt + 1], scale=1.0)
            return outT3

        gamT3 = mlp_head(wg1_b3, bg1_s, wg2_b, bg2p1_s, "g")
        betT3 = mlp_head(wb1_b3, bb1_s, wb2_b, bb2_s, "b")

        # ---- es = 10*gam*rs ; wb' = 10*bet - es*mu ----
        es = const.tile([128, NDT * BI], F32, tag="es")
        wb = const.tile([128, NDT * BI], F32, tag="wb")
        es3 = es[:].rearrange("p (c i) -> p c i", c=NDT)
        wb3 = wb[:].rearrange("p (c i) -> p c i", c=NDT)
        rs_b = rs[:].rearrange("p (c u) -> p c u", u=1).broadcast_to(
            [128, NDT, BI])
        mu_b = mu[:].rearrange("p (c u) -> p c u", u=1).broadcast_to(
            [128, NDT, BI])
        tmp64 = small.tile([128, NDT * BI], F32, tag="tmp64", bufs=2)
        tmp3 = tmp64[:].rearrange("p (c i) -> p c i", c=NDT)
        nc.vector.tensor_tensor(es3, gamT3, rs_b, op=ALU.mult)
        nc.vector.tensor_scalar_mul(es[:], es[:], 10.0)
        nc.vector.tensor_tensor(tmp3, es3, mu_b, op=ALU.mult)
        nc.vector.tensor_scalar_mul(wb[:], betT3.rearrange("p c i -> p (c i)"),
                                    10.0)
        nc.vector.tensor_tensor(wb[:], wb[:], tmp64[:], op=ALU.subtract)

        # ---- main loop ----
        actx.close()  # release phase psum banks
        pmain = ctx.enter_context(tc.tile_pool(name="pmain", bufs=1,
                                               space="PSUM"))
        psacc = ctx.enter_context(tc.tile_pool(name="psacc", bufs=2,
                                               space="PSUM"))
        ptr = None
        if _xj < NPT:
            ptr = ctx.enter_context(tc.tile_pool(name="ptr", bufs=2,
                                                 space="PSUM"))
        dot_ps = pmain.tile([1, BI * B_CAP], F32, tag="dot_ps")
        nrm_ps = pmain.tile([1, BI * B_CAP], F32, tag="nrm_ps")

        for _rep in range(reps_main):
            for i in range(BI):
                s12_ps = psacc.tile([128, 2 * NDT * B_CAP], F32, tag="s12")
                s12v = s12_ps[:].rearrange("p (s c b) -> p s c b", s=2, c=NDT)
                for dt in range(NDT):
                    e_t = work.tile([128, R], BF16, tag="e")
                    nc.scalar.activation(e_t[:], capT3[:, dt, :], AF.Exp,
                                         bias=0.0, scale=es3[:, dt, i:i + 1])
                    eT = work.tile([128, NPT * 128], BF16, tag="eT")
                    eT3 = eT[:].rearrange("p (c j) -> p c j", c=NPT)
                    if _xj > 0:
                        nc.sync.dma_start_transpose(eT3[:, 0:_xj, :],
                                                    e_t[:, 0:_xj * 128])
                    if _xj < NPT:
                        npe = NPT - _xj
                        tr_ps = ptr.tile([128, npe * 128], BF16, tag="tr")
                        for k in range(npe):
                            nc.tensor.transpose(
                                tr_ps[:, k * 128:(k + 1) * 128],
                                e_t[:, (_xj + k) * 128:(_xj + k + 1) * 128],
                                ident_b[:])
                        nc.vector.tensor_copy(eT[:, _xj * 128:NPT * 128],
                                              tr_ps[:])
                    qT = work.tile([128, NPT * 128], BF16, tag="qT")
                    qT3 = qT[:].rearrange("p (c j) -> p c j", c=NPT)
                    nc.vector.tensor_tensor(
                        qT3, eT3, capR3[:, :, dt * 128:(dt + 1) * 128],
                        op=ALU.mult)
                    for c in range(NPT):
                        nc.tensor.matmul(s12v[:, 0, dt, :], lhsT=eT3[:, c, :],
                                         rhs=ind3[:, c, :],
                                         start=(c == 0), stop=(c == NPT - 1))
                    for c in range(NPT):
                        nc.tensor.matmul(s12v[:, 1, dt, :], lhsT=qT3[:, c, :],
                                         rhs=ind3[:, c, :],
                                         start=(c == 0), stop=(c == NPT - 1))
                # sc = s2/s1 ; w' = es*sc + wb' ; dots on PE
                r1 = work.tile([128, NDT * B_CAP], F32, tag="r1", bufs=2)
                nc.vector.reciprocal(r1[:], s12_ps[:, 0:NDT * B_CAP])
                sc = work.tile([128, NDT * B_CAP], F32, tag="sc", bufs=2)
                nc.vector.tensor_tensor(
                    sc[:], s12_ps[:, NDT * B_CAP:2 * NDT * B_CAP], r1[:],
                    op=ALU.mult)
                for dt in range(NDT):
                    w_t = work.tile([128, B_CAP], BF16, tag="w")
                    nc.vector.tensor_scalar(
                        w_t[:], sc[:, dt * B_CAP:(dt + 1) * B_CAP],
                        es3[:, dt, i:i + 1], wb3[:, dt, i:i + 1],
                        op0=ALU.mult, op1=ALU.add)
                    w2_t = work.tile([128, B_CAP], BF16, tag="w2")
                    nc.gpsimd.tensor_tensor(w2_t[:], w_t[:], w_t[:],
                                            op=ALU.mult)
                    nc.tensor.matmul(dot_ps[:, i * B_CAP:(i + 1) * B_CAP],
                                     lhsT=imgrTb3[:, dt, i:i + 1], rhs=w_t[:],
                                     start=(dt == 0), stop=(dt == NDT - 1))
                    nc.tensor.matmul(nrm_ps[:, i * B_CAP:(i + 1) * B_CAP],
                                     lhsT=ones_b[:], rhs=w2_t[:],
                                     start=(dt == 0), stop=(dt == NDT - 1))

            # ---- epilogue: sims = dot * Exp(-0.5*Ln(nrm)) * (1/|v|) ----
            rsn = small.tile([1, BI * B_CAP], F32, tag="rsn")
            nc.scalar.activation(rsn[:], nrm_ps[:], AF.Ln)
            nc.scalar.activation(rsn[:], rsn[:], AF.Exp, scale=-0.5)
            prod = small.tile([1, BI * B_CAP], F32, tag="prod")
            nc.vector.tensor_tensor(prod[:], dot_ps[:], rsn[:], op=ALU.mult)
            res = small.tile([1, BI * B_CAP], F32, tag="res")
            rsr_b = rsr_row[:].rearrange("p (i u) -> p i u", u=1).broadcast_to(
                [1, BI, B_CAP])
            nc.vector.tensor_tensor(
                res[:].rearrange("p (i b) -> p i b", i=BI),
                prod[:].rearrange("p (i b) -> p i b", i=BI),
                rsr_b, op=ALU.mult)
            nc.sync.dma_start(out=out_d[:].rearrange("i b -> (i b)"),
                              in_=res[:])


def _build():
    nc = bacc.Bacc("TRN2", target_bir_lowering=False, debug=False,
                   num_devices=N_CORES)
    tensors = _declare_io(nc)
    with TileContext(nc) as tc:
        _emit(nc, tc, *tensors)
    nc.compile()
    return nc


def _build_repeated(reps):
    """Timing variant: run the compute `reps` times in one NEFF. With
    KREPMODE=main, phase A runs once and only the main loop repeats."""
    nc = bacc.Bacc("TRN2", target_bir_lowering=False, debug=False,
                   num_devices=N_CORES)
    tensors = _declare_io(nc)
    with TileContext(nc) as tc:
        if os.environ.get("KREPMODE") == "main":
            _emit(nc, tc, *tensors, reps_main=reps)
        else:
            for _ in range(reps):
                _emit(nc, tc, *tensors)
    nc.compile()
    return nc


def _get_compiled():
    global _COMPILED
    if _COMPILED is None:
        _COMPILED = _build()
    return _COMPILED


def _indicator():
    ind = np.zeros((128, NPT, B_CAP), np.float32)
    for c in range(NPT):
        for r in range(128):
            ind[r, c, (c * 128 + r) // T_IMG] = 1.0
    return ind.reshape(128, NPT * B_CAP)


def _in_maps(img_embed, cap_embed, Wg1, bg1, Wg2, bg2, Wb1, bb1, Wb2, bb2):
    cap = np.ascontiguousarray(
        cap_embed[:, :T_IMG, :].reshape(R, D)).astype(np.float32)

    def w1_tiles(W):
        return np.ascontiguousarray(
            W.reshape(NDT, 128, H).transpose(1, 0, 2).reshape(128, NDT * H)
        ).astype(BF16_NP)

    shared = {
        "capt": np.ascontiguousarray(cap.T).astype(BF16_NP),
        "capr": cap.astype(BF16_NP),
        "ind": _indicator().astype(BF16_NP),
        "wg1b": w1_tiles(np.asarray(Wg1, np.float32)),
        "wb1b": w1_tiles(np.asarray(Wb1, np.float32)),
        "wg2b": np.ascontiguousarray(Wg2, np.float32).astype(BF16_NP),
        "wb2b": np.ascontiguousarray(Wb2, np.float32).astype(BF16_NP),
        "bg1": np.ascontiguousarray(bg1.reshape(H, 1), np.float32),
        "bb1": np.ascontiguousarray(bb1.reshape(H, 1), np.float32),
        "bg2p1": np.ascontiguousarray((bg2 + 1.0).reshape(NDT, 128).T,
                                      np.float32),
        "bb2t": np.ascontiguousarray(bb2.reshape(NDT, 128).T, np.float32),
        "o36": np.full((T_IMG, 1), 1.0 / T_IMG, np.float32),
    }
    maps = []
    for c in range(N_CORES):
        m = dict(shared)
        m["img"] = np.ascontiguousarray(
            img_embed[c * BI:(c + 1) * BI], np.float32)
        maps.append(m)
    return maps


def kernel(img_embed, cap_embed, lens, Wg1, bg1, Wg2, bg2, Wb1, bb1, Wb2, bb2):
    del lens  # unused by the reference computation
    nc = _get_compiled()
    maps = _in_maps(np.asarray(img_embed), np.asarray(cap_embed),
                    np.asarray(Wg1), np.asarray(bg1), np.asarray(Wg2),
                    np.asarray(bg2), np.asarray(Wb1), np.asarray(bb1),
                    np.asarray(Wb2), np.asarray(bb2))
    import time as _time
    last = None
    for attempt in range(5):  # device occasionally needs runs to recover
        try:
            res = run_bass_kernel_spmd(nc, maps, core_ids=list(range(N_CORES)))
            break
        except Exception as e:
            last = e
            _time.sleep(10)
    else:
        raise last
    return np.concatenate([res.results[c]["out"] for c in range(N_CORES)],
                          axis=0).astype(np.float32)


# revision 34
# speedup vs baseline: 1.3586x; 1.2124x over previous
"""Trainium2 Bass kernel for nn_AdaptiveEmbeddingI2T.

Computes, for image-batch shard i on each of 8 NeuronCores:
  sims[i, b] = <img_vec_i, txt_vec_ib> with
  txt_vec_ib = l2norm_d( mean_t( softmax_t(10*(gam_id*xn_bdt+bet_id)) * (gam*xn+bet) ) )

Device-side algebra (per image i, channel d, caption b, time t):
  - softmax over t is shift/scale invariant in the ratio
      sc[d,b] = sum_t(e*cap) / sum_t(e),  e = exp(es[d,i]*cap[d,b,t]),
      es = 10*gam*rs   (the -es*mu shift and exp(bias) factor cancel)
  - txt_vec ~ w' = es*sc + (10*bet - es*mu)   (any uniform scale of w'
      cancels in the final l2 normalization, so the /36 and /10 drop)
  - sims = (sum_d v*w') * rsqrt(sum_d w'^2) * rsqrt(sum_d v^2)

Engine mapping:
  - exp on ACT in [d-partition, (b t)] layout (per-partition scale port)
  - e -> eT: one DMA-XBAR transpose for the first KXJ row-chunks, PE
    transposes + DVE psum->sbuf copy for the rest (balances DMA vs PE/DVE)
  - qT = eT * capR elementwise on DVE (the big DVE op)
  - s1 = sum_t e and s2 = sum_t q as PE indicator matmuls over the
    r=(b,t) partition chunks, accumulating f32 in PSUM (no DVE trees)
  - w' affine on DVE (tensor_scalar), w'^2 on GPSIMD, dots on PE
  - BN stats via PE matmuls (ones / self) off capR, diag extract on DVE
  - all rsqrt via Exp(-0.5*Ln(x)) so every ACT func lives in the single
    natural_log_exp_and_others table (no act-table reloads)

Sharding: image batch axis across 8 cores (8 images/core); cap + params
replicated; host concatenates the (8, 64) row blocks.
"""

import os
import sys

if "/opt/trn_rl_repo" not in sys.path:
    sys.path.insert(0, "/opt/trn_rl_repo")

import numpy as np
import ml_dtypes

import concourse.bacc as bacc
import concourse.mybir as mybir
from concourse import masks
from concourse.bass_utils import run_bass_kernel_spmd
from concourse.tile import TileContext

B_IMG, B_CAP, T_CAP, D = 64, 64, 64, 1024
H = 128
T_IMG = 36
EPS = 1e-5
N_CORES = 8
BI = B_IMG // N_CORES          # images per core
R = B_CAP * T_IMG              # 2304 caption rows
NPT = R // 128                 # 18 caption row chunks
NDT = D // 128                 # 8 channel tiles

F32 = mybir.dt.float32
BF16 = mybir.dt.bfloat16
BF16_NP = ml_dtypes.bfloat16

AF = mybir.ActivationFunctionType
ALU = mybir.AluOpType
AX = mybir.AxisListType

_COMPILED = None


def _declare_io(nc):
    return (
        nc.dram_tensor("capt", [D, R], BF16, kind="ExternalInput"),
        nc.dram_tensor("capr", [R, D], BF16, kind="ExternalInput"),
        nc.dram_tensor("ind", [128, NPT * B_CAP], BF16, kind="ExternalInput"),
        nc.dram_tensor("img", [BI, T_IMG, D], F32, kind="ExternalInput"),
        nc.dram_tensor("wg1b", [128, NDT * H], BF16, kind="ExternalInput"),
        nc.dram_tensor("wg2b", [H, D], BF16, kind="ExternalInput"),
        nc.dram_tensor("wb1b", [128, NDT * H], BF16, kind="ExternalInput"),
        nc.dram_tensor("wb2b", [H, D], BF16, kind="ExternalInput"),
        nc.dram_tensor("bg1", [H, 1], F32, kind="ExternalInput"),
        nc.dram_tensor("bb1", [H, 1], F32, kind="ExternalInput"),
        nc.dram_tensor("bg2p1", [128, NDT], F32, kind="ExternalInput"),
        nc.dram_tensor("bb2t", [128, NDT], F32, kind="ExternalInput"),
        nc.dram_tensor("o36", [T_IMG, 1], F32, kind="ExternalInput"),
        nc.dram_tensor("out", [BI, B_CAP], F32, kind="ExternalOutput"),
    )


def _emit(nc, tc, capt_d, capr_d, ind_d, img_d, wg1_d, wg2_d, wb1_d, wb2_d,
          bg1_d, bb1_d, bg2p1_d, bb2_d, o36_d, out_d, reps_main=1):
    import contextlib
    ctx = contextlib.ExitStack()
    _xj = int(os.environ.get("KXJ", "12"))        # chunks via XBAR
    _cpact = os.environ.get("KCPACT", "0") == "1"  # alternate copies on ACT
    with ctx:
        const = ctx.enter_context(tc.tile_pool(name="const", bufs=2))
        capx = ctx.enter_context(tc.tile_pool(name="capx", bufs=1))
        imgs = ctx.enter_context(tc.tile_pool(name="imgs", bufs=2))
        work = ctx.enter_context(tc.tile_pool(name="work", bufs=3))
        small = ctx.enter_context(tc.tile_pool(name="small", bufs=1))
        actx = ctx.enter_context(contextlib.ExitStack())
        ppool = actx.enter_context(tc.tile_pool(name="psum", bufs=1,
                                                space="PSUM"))
        pacc = actx.enter_context(tc.tile_pool(name="pacc", bufs=1,
                                               space="PSUM"))

        ident = const.tile([128, 128], F32, bufs=1)
        masks.make_identity(nc, ident[:])
        ident_b = const.tile([128, 128], BF16, tag="identb", bufs=1)
        masks.make_identity(nc, ident_b[:])
        ones_b = const.tile([128, 1], BF16, tag="onesb", bufs=1)
        nc.gpsimd.memset(ones_b[:], 1.0)

        # ---- loads: smalls/weights/img first, then capR (stats), capT ----
        bg1_s = const.tile([H, 1], F32, tag="bg1", bufs=1)
        nc.sync.dma_start(out=bg1_s[:], in_=bg1_d[:])
        bb1_s = const.tile([H, 1], F32, tag="bb1", bufs=1)
        nc.sync.dma_start(out=bb1_s[:], in_=bb1_d[:])
        bg2p1_s = const.tile([128, NDT], F32, tag="bg2p1", bufs=1)
        nc.sync.dma_start(out=bg2p1_s[:], in_=bg2p1_d[:])
        bb2_s = const.tile([128, NDT], F32, tag="bb2t", bufs=1)
        nc.sync.dma_start(out=bb2_s[:], in_=bb2_d[:])
        o36_s = const.tile([T_IMG, 1], F32, tag="o36", bufs=1)
        nc.sync.dma_start(out=o36_s[:], in_=o36_d[:])
        wg1_b = const.tile([128, NDT * H], BF16, tag="wg1b", bufs=1)
        nc.sync.dma_start(out=wg1_b[:], in_=wg1_d[:])
        wb1_b = const.tile([128, NDT * H], BF16, tag="wb1b", bufs=1)
        nc.sync.dma_start(out=wb1_b[:], in_=wb1_d[:])
        wg2_b = const.tile([128, D], BF16, tag="wg2b", bufs=1)
        nc.sync.dma_start(out=wg2_b[:], in_=wg2_d[:])
        wb2_b = const.tile([128, D], BF16, tag="wb2b", bufs=1)
        nc.sync.dma_start(out=wb2_b[:], in_=wb2_d[:])
        img_tiles = []
        for i in range(BI):
            ichunk = imgs.tile([T_IMG, D], F32, tag="ichunk", bufs=4)
            nc.sync.dma_start(out=ichunk[:], in_=img_d[i])
            img_tiles.append(ichunk)

        ind_s = capx.tile([128, NPT * B_CAP], BF16, tag="ind", bufs=2)
        ind3 = ind_s[:].rearrange("p (c b) -> p c b", c=NPT)
        nc.sync.dma_start(out=ind_s[:], in_=ind_d[:])
        capR = capx.tile([128, NPT * D], BF16, tag="capR", bufs=2)
        capR3 = capR[:].rearrange("p (c d) -> p c d", c=NPT)
        capr_v = capr_d[:].rearrange("(c p) d -> p c d", p=128)
        for c in range(NPT):
            nc.sync.dma_start(out=capR3[:, c, :], in_=capr_v[:, c, :])
        capT = capx.tile([128, NDT * R], BF16, tag="capT", bufs=1)
        capT3 = capT[:].rearrange("p (c r) -> p c r", c=NDT)
        capt_v = capt_d[:].rearrange("(c p) r -> p c r", p=128)
        for dt in range(NDT):
            nc.sync.dma_start(out=capT3[:, dt, :], in_=capt_v[:, dt, :])

        # ---- BN stats on PE off capR chunks (chunk-major) ----
        mus_ps = pacc.tile([128, NDT], F32, tag="mus_ps")
        for c in range(NPT):
            for dt in range(NDT):
                nc.tensor.matmul(mus_ps[:, dt:dt + 1],
                                 lhsT=capR3[:, c, dt * 128:(dt + 1) * 128],
                                 rhs=ones_b[:],
                                 start=(c == 0), stop=(c == NPT - 1))
        sqsum = small.tile([128, NDT], F32, tag="sqsum", bufs=2)
        sq_tiles = []
        for k in range(NDT):
            sq_t = ppool.tile([128, 128], F32, tag=f"sq_ps{k % 3}")
            sq_tiles.append(sq_t)
        for wave, dts in enumerate((range(0, 3), range(3, 6), range(6, 8))):
            for c in range(NPT):
                for dt in dts:
                    nc.tensor.matmul(
                        sq_tiles[dt][:],
                        lhsT=capR3[:, c, dt * 128:(dt + 1) * 128],
                        rhs=capR3[:, c, dt * 128:(dt + 1) * 128],
                        start=(c == 0), stop=(c == NPT - 1))
            for dt in dts:
                dg = work.tile([128, 128], F32, tag="dg", bufs=2)
                nc.vector.tensor_tensor(dg[:], sq_tiles[dt][:], ident[:],
                                        op=ALU.mult)
                nc.vector.tensor_reduce(
                    sqsum[:, dt:dt + 1],
                    dg[:].rearrange("p (u q) -> p u q", u=1),
                    axis=AX.X, op=ALU.add)

        # mu = musum/R ; var = E[x^2]-mu^2 ; rs = Exp(-0.5*Ln(var+eps))
        mu = small.tile([128, NDT], F32, tag="mu", bufs=2)
        rs = small.tile([128, NDT], F32, tag="rs", bufs=2)
        tv = small.tile([128, NDT], F32, tag="tv", bufs=2)
        nc.vector.tensor_scalar_mul(mu[:], mus_ps[:], 1.0 / R)
        nc.vector.tensor_tensor(tv[:], mu[:], mu[:], op=ALU.mult)
        nc.vector.tensor_scalar(sqsum[:], sqsum[:], 1.0 / R, None,
                                op0=ALU.mult)
        nc.vector.tensor_tensor(tv[:], sqsum[:], tv[:], op=ALU.subtract)
        nc.vector.tensor_scalar_add(tv[:], tv[:], EPS)
        nc.scalar.activation(tv[:], tv[:], AF.Ln)
        nc.scalar.activation(rs[:], tv[:], AF.Exp, scale=-0.5)

        # ---- image means, directly transposed: imgrT [128, (dt i)] ----
        imgrT_ps = pacc.tile([128, NDT * BI], F32, tag="imgrT_ps")
        for i in range(BI):
            ichunk = img_tiles[i]
            for dt in range(NDT):
                nc.tensor.matmul(
                    imgrT_ps[:, dt * BI + i:dt * BI + i + 1],
                    lhsT=ichunk[:, dt * 128:(dt + 1) * 128], rhs=o36_s[:],
                    start=True, stop=True)

        imgrT = const.tile([128, NDT * BI], F32, tag="imgrT")
        imgrTb = const.tile([128, NDT * BI], BF16, tag="imgrTb")
        imgrT3 = imgrT[:].rearrange("p (c i) -> p c i", c=NDT)
        imgrTb3 = imgrTb[:].rearrange("p (c i) -> p c i", c=NDT)
        nc.vector.tensor_copy(imgrT[:], imgrT_ps[:])
        nc.scalar.copy(imgrTb[:], imgrT_ps[:])

        # 1/||v_i|| via accumulating [1,1] matmuls, rsqrt via Ln/Exp
        nrm2_ps = pacc.tile([1, BI], F32, tag="nrm2_ps")
        for i in range(BI):
            for dt in range(NDT):
                nc.tensor.matmul(
                    nrm2_ps[:, i:i + 1],
                    lhsT=imgrT3[:, dt, i:i + 1], rhs=imgrT3[:, dt, i:i + 1],
                    start=(dt == 0), stop=(dt == NDT - 1))
        rsr_row = small.tile([1, BI], F32, tag="rsr_row", bufs=2)
        nc.scalar.activation(rsr_row[:], nrm2_ps[:], AF.Ln)
        nc.scalar.activation(rsr_row[:], rsr_row[:], AF.Exp, scale=-0.5)

        # ---- CBN MLPs -> gamT/betT [128, (dt, i)] f32 ----
        wg1_b3 = wg1_b[:].rearrange("p (c h) -> p c h", c=NDT)
        wb1_b3 = wb1_b[:].rearrange("p (c h) -> p c h", c=NDT)

        def mlp_head(w1_b3, b1_s, w2_b, b2_s, name):
            h_ps = ppool.tile([H, BI], F32, tag="h_ps")
            for dt in range(NDT):
                nc.tensor.matmul(h_ps[:], lhsT=w1_b3[:, dt, :],
                                 rhs=imgrTb3[:, dt, :],
                                 start=(dt == 0), stop=(dt == NDT - 1))
            hT = small.tile([H, BI], BF16, tag=f"hT_{name}", bufs=2)
            nc.scalar.activation(hT[:], h_ps[:], AF.Relu, bias=b1_s[:],
                                 scale=1.0)
            outT = const.tile([128, NDT * BI], F32, tag=f"outT_{name}")
            outT3 = outT[:].rearrange("p (c i) -> p c i", c=NDT)
            for dt in range(NDT):
                o_ps = ppool.tile([128, BI], F32, tag="o_ps")
                nc.tensor.matmul(o_ps[:],
                                 lhsT=w2_b[:, dt * 128:(dt + 1) * 128],
                                 rhs=hT[:], start=True, stop=True)
                nc.scalar.activation(outT3[:, dt, :], o_ps[:], AF.Identity,
                                     bias=b2_s[:, dt:dt + 1], scale=1.0)
            return outT3

        gamT3 = mlp_head(wg1_b3, bg1_s, wg2_b, bg2p1_s, "g")
        betT3 = mlp_head(wb1_b3, bb1_s, wb2_b, bb2_s, "b")

        # ---- es = 10*gam*rs ; wb' = 10*bet - es*mu ----
        es = const.tile([128, NDT * BI], F32, tag="es")
        wb = const.tile([128, NDT * BI], F32, tag="wb")
        es3 = es[:].rearrange("p (c i) -> p c i", c=NDT)
        wb3 = wb[:].rearrange("p (c i) -> p c i", c=NDT)
        rs_b = rs[:].rearrange("p (c u) -> p c u", u=1).broadcast_to(
            [128, NDT, BI])
        mu_b = mu[:].rearrange("p (c u) -> p c u", u=1).broadcast_to(
            [128, NDT, BI])
        tmp64 = small.tile([128, NDT * BI], F32, tag="tmp64", bufs=2)
        tmp3 = tmp64[:].rearrange("p (c i) -> p c i", c=NDT)
        nc.vector.tensor_tensor(es3, gamT3, rs_b, op=ALU.mult)
        nc.vector.tensor_scalar_mul(es[:], es[:], 10.0)
        nc.vector.tensor_tensor(tmp3, es3, mu_b, op=ALU.mult)
        nc.vector.tensor_scalar_mul(wb[:], betT3.rearrange("p c i -> p (c i)"),
                                    10.0)
        nc.vector.tensor_tensor(wb[:], wb[:], tmp64[:], op=ALU.subtract)

        # ---- main loop ----
        actx.close()  # release phase psum banks
        pmain = ctx.enter_context(tc.tile_pool(name="pmain", bufs=1,
                                               space="PSUM"))
        psacc = ctx.enter_context(tc.tile_pool(name="psacc", bufs=2,
                                               space="PSUM"))
        ptr = None
        if _xj < NPT:
            ptr = ctx.enter_context(tc.tile_pool(name="ptr", bufs=2,
                                                 space="PSUM"))
        dot_ps = pmain.tile([1, BI * B_CAP], F32, tag="dot_ps")
        nrm_ps = pmain.tile([1, BI * B_CAP], F32, tag="nrm_ps")

        for _rep in range(reps_main):
            for i in range(BI):
                s12_ps = psacc.tile([128, 2 * NDT * B_CAP], F32, tag="s12")
                s12v = s12_ps[:].rearrange("p (s c b) -> p s c b", s=2, c=NDT)
                for dt in range(NDT):
                    e_t = work.tile([128, R], BF16, tag="e")
                    nc.scalar.activation(e_t[:], capT3[:, dt, :], AF.Exp,
                                         bias=0.0, scale=es3[:, dt, i:i + 1])
                    eT = work.tile([128, NPT * 128], BF16, tag="eT")
                    eT3 = eT[:].rearrange("p (c j) -> p c j", c=NPT)
                    if _xj > 0:
                        nc.sync.dma_start_transpose(eT3[:, 0:_xj, :],
                                                    e_t[:, 0:_xj * 128])
                    if _xj < NPT:
                        npe = NPT - _xj
                        done = 0
                        wv = 0
                        while done < npe:
                            n_w = min(8, npe - done)
                            tr_ps = ptr.tile([128, n_w * 128], BF16,
                                             tag="tr")
                            for k in range(n_w):
                                src = _xj + done + k
                                nc.tensor.transpose(
                                    tr_ps[:, k * 128:(k + 1) * 128],
                                    e_t[:, src * 128:(src + 1) * 128],
                                    ident_b[:])
                            dst = eT[:, (_xj + done) * 128:
                                     (_xj + done + n_w) * 128]
                            if _cpact and wv % 2 == 1:
                                nc.scalar.copy(dst, tr_ps[:])
                            else:
                                nc.vector.tensor_copy(dst, tr_ps[:])
                            done += n_w
                            wv += 1
                    qT = work.tile([128, NPT * 128], BF16, tag="qT")
                    qT3 = qT[:].rearrange("p (c j) -> p c j", c=NPT)
                    nc.vector.tensor_tensor(
                        qT3, eT3, capR3[:, :, dt * 128:(dt + 1) * 128],
                        op=ALU.mult)
                    for c in range(NPT):
                        nc.tensor.matmul(s12v[:, 0, dt, :], lhsT=eT3[:, c, :],
                                         rhs=ind3[:, c, :],
                                         start=(c == 0), stop=(c == NPT - 1))
                    for c in range(NPT):
                        nc.tensor.matmul(s12v[:, 1, dt, :], lhsT=qT3[:, c, :],
                                         rhs=ind3[:, c, :],
                                         start=(c == 0), stop=(c == NPT - 1))
                # sc = s2/s1 ; w' = es*sc + wb' ; dots on PE
                r1 = work.tile([128, NDT * B_CAP], F32, tag="r1", bufs=2)
                nc.vector.reciprocal(r1[:], s12_ps[:, 0:NDT * B_CAP])
                sc = work.tile([128, NDT * B_CAP], F32, tag="sc", bufs=2)
                nc.vector.tensor_tensor(
                    sc[:], s12_ps[:, NDT * B_CAP:2 * NDT * B_CAP], r1[:],
                    op=ALU.mult)
                for dt in range(NDT):
                    w_t = work.tile([128, B_CAP], BF16, tag="w")
                    nc.vector.tensor_scalar(
                        w_t[:], sc[:, dt * B_CAP:(dt + 1) * B_CAP],
                        es3[:, dt, i:i + 1], wb3[:, dt, i:i + 1],
                        op0=ALU.mult, op1=ALU.add)
                    w2_t = work.tile([128, B_CAP], BF16, tag="w2")
                    nc.gpsimd.tensor_tensor(w2_t[:], w_t[:], w_t[:],
                                            op=ALU.mult)
                    nc.tensor.matmul(dot_ps[:, i * B_CAP:(i + 1) * B_CAP],
                                     lhsT=imgrTb3[:, dt, i:i + 1], rhs=w_t[:],
                                     start=(dt == 0), stop=(dt == NDT - 1))
                    nc.tensor.matmul(nrm_ps[:, i * B_CAP:(i + 1) * B_CAP],
                                     lhsT=ones_b[:], rhs=w2_t[:],
                                     start=(dt == 0), stop=(dt == NDT - 1))

            # ---- epilogue: sims = dot * Exp(-0.5*Ln(nrm)) * (1/|v|) ----
            rsn = small.tile([1, BI * B_CAP], F32, tag="rsn")
            nc.scalar.activation(rsn[:], nrm_ps[:], AF.Ln)
            nc.scalar.activation(rsn[:], rsn[:], AF.Exp, scale=-0.5)
            prod = small.tile([1, BI * B_CAP], F32, tag="prod")
            nc.vector.tensor_tensor(prod[:], dot_ps[:], rsn[:], op=ALU.mult)
            res = small.tile([1, BI * B_CAP], F32, tag="res")
            rsr_b = rsr_row[:].rearrange("p (i u) -> p i u", u=1).broadcast_to(
                [1, BI, B_CAP])
            nc.vector.tensor_tensor(
                res[:].rearrange("p (i b) -> p i b", i=BI),
                prod[:].rearrange("p (i b) -> p i b", i=BI),
                rsr_b, op=ALU.mult)
            nc.sync.dma_start(out=out_d[:].rearrange("i b -> (i b)"),
                              in_=res[:])


def _build():
    nc = bacc.Bacc("TRN2", target_bir_lowering=False, debug=False,
                   num_devices=N_CORES)
    tensors = _declare_io(nc)
    with TileContext(nc) as tc:
        _emit(nc, tc, *tensors)
    nc.compile()
    return nc


def _build_repeated(reps):
    """Timing variant: run the compute `reps` times in one NEFF. With
    KREPMODE=main, phase A runs once and only the main loop repeats."""
    nc = bacc.Bacc("TRN2", target_bir_lowering=False, debug=False,
                   num_devices=N_CORES)
    tensors = _declare_io(nc)
    with TileContext(nc) as tc:
        if os.environ.get("KREPMODE") == "main":
            _emit(nc, tc, *tensors, reps_main=reps)
        else:
            for _ in range(reps):
                _emit(nc, tc, *tensors)
    nc.compile()
    return nc


def _get_compiled():
    global _COMPILED
    if _COMPILED is None:
        _COMPILED = _build()
    return _COMPILED


def _indicator():
    ind = np.zeros((128, NPT, B_CAP), np.float32)
    for c in range(NPT):
        for r in range(128):
            ind[r, c, (c * 128 + r) // T_IMG] = 1.0
    return ind.reshape(128, NPT * B_CAP)


def _in_maps(img_embed, cap_embed, Wg1, bg1, Wg2, bg2, Wb1, bb1, Wb2, bb2):
    cap = np.ascontiguousarray(
        cap_embed[:, :T_IMG, :].reshape(R, D)).astype(np.float32)

    def w1_tiles(W):
        return np.ascontiguousarray(
            W.reshape(NDT, 128, H).transpose(1, 0, 2).reshape(128, NDT * H)
        ).astype(BF16_NP)

    shared = {
        "capt": np.ascontiguousarray(cap.T).astype(BF16_NP),
        "capr": cap.astype(BF16_NP),
        "ind": _indicator().astype(BF16_NP),
        "wg1b": w1_tiles(np.asarray(Wg1, np.float32)),
        "wb1b": w1_tiles(np.asarray(Wb1, np.float32)),
        "wg2b": np.ascontiguousarray(Wg2, np.float32).astype(BF16_NP),
        "wb2b": np.ascontiguousarray(Wb2, np.float32).astype(BF16_NP),
        "bg1": np.ascontiguousarray(bg1.reshape(H, 1), np.float32),
        "bb1": np.ascontiguousarray(bb1.reshape(H, 1), np.float32),
        "bg2p1": np.ascontiguousarray((bg2 + 1.0).reshape(NDT, 128).T,
                                      np.float32),
        "bb2t": np.ascontiguousarray(bb2.reshape(NDT, 128).T, np.float32),
        "o36": np.full((T_IMG, 1), 1.0 / T_IMG, np.float32),
    }
    maps = []
    for c in range(N_CORES):
        m = dict(shared)
        m["img"] = np.ascontiguousarray(
            img_embed[c * BI:(c + 1) * BI], np.float32)
        maps.append(m)
    return maps


def kernel(img_embed, cap_embed, lens, Wg1, bg1, Wg2, bg2, Wb1, bb1, Wb2, bb2):
    del lens  # unused by the reference computation
    nc = _get_compiled()
    maps = _in_maps(np.asarray(img_embed), np.asarray(cap_embed),
                    np.asarray(Wg1), np.asarray(bg1), np.asarray(Wg2),
                    np.asarray(bg2), np.asarray(Wb1), np.asarray(bb1),
                    np.asarray(Wb2), np.asarray(bb2))
    import time as _time
    last = None
    for attempt in range(5):  # device occasionally needs runs to recover
        try:
            res = run_bass_kernel_spmd(nc, maps, core_ids=list(range(N_CORES)))
            break
        except Exception as e:
            last = e
            _time.sleep(10)
    else:
        raise last
    return np.concatenate([res.results[c]["out"] for c in range(N_CORES)],
                          axis=0).astype(np.float32)


# revision 35
# speedup vs baseline: 1.5213x; 1.1198x over previous
"""Trainium2 Bass kernel for nn_AdaptiveEmbeddingI2T.

Computes, for image-batch shard i on each of 8 NeuronCores:
  sims[i, b] = <img_vec_i, txt_vec_ib> with
  txt_vec_ib = l2norm_d( mean_t( softmax_t(10*(gam_id*xn_bdt+bet_id)) * (gam*xn+bet) ) )

Device-side algebra (per image i, channel d, caption b, time t):
  - softmax over t is shift/scale invariant in the ratio
      sc[d,b] = sum_t(e*cap) / sum_t(e),  e = exp(es[d,i]*cap[d,b,t]),
      es = 10*gam*rs   (the -es*mu shift and exp(bias) factor cancel)
  - txt_vec ~ w' = es*sc + (10*bet - es*mu)   (any uniform scale of w'
      cancels in the final l2 normalization, so the /36 and /10 drop)
  - sims = (sum_d v*w') * rsqrt(sum_d w'^2) * rsqrt(sum_d v^2)

Engine mapping:
  - exp on ACT in [d-partition, (b t)] layout (per-partition scale port)
  - e -> eT: one DMA-XBAR transpose for the first KXJ row-chunks, PE
    transposes + DVE psum->sbuf copy for the rest (balances DMA vs PE/DVE)
  - qT = eT * capR elementwise on DVE (the big DVE op)
  - s1 = sum_t e and s2 = sum_t q as PE indicator matmuls over the
    r=(b,t) partition chunks, accumulating f32 in PSUM (no DVE trees)
  - w' affine on DVE (tensor_scalar), w'^2 on GPSIMD, dots on PE
  - BN stats via PE matmuls (ones / self) off capR, diag extract on DVE
  - all rsqrt via Exp(-0.5*Ln(x)) so every ACT func lives in the single
    natural_log_exp_and_others table (no act-table reloads)

Sharding: image batch axis across 8 cores (8 images/core); cap + params
replicated; host concatenates the (8, 64) row blocks.
"""

import os
import sys

if "/opt/trn_rl_repo" not in sys.path:
    sys.path.insert(0, "/opt/trn_rl_repo")

import numpy as np
import ml_dtypes

import concourse.bacc as bacc
import concourse.mybir as mybir
from concourse import masks
from concourse.bass_utils import run_bass_kernel_spmd
from concourse.tile import TileContext

B_IMG, B_CAP, T_CAP, D = 64, 64, 64, 1024
H = 128
T_IMG = 36
EPS = 1e-5
N_CORES = 8
BI = B_IMG // N_CORES          # images per core
R = B_CAP * T_IMG              # 2304 caption rows
NPT = R // 128                 # 18 caption row chunks
NDT = D // 128                 # 8 channel tiles

F32 = mybir.dt.float32
BF16 = mybir.dt.bfloat16
BF16_NP = ml_dtypes.bfloat16

AF = mybir.ActivationFunctionType
ALU = mybir.AluOpType
AX = mybir.AxisListType

_COMPILED = None


def _declare_io(nc):
    return (
        nc.dram_tensor("capt", [D, R], BF16, kind="ExternalInput"),
        nc.dram_tensor("capr", [R, D], BF16, kind="ExternalInput"),
        nc.dram_tensor("ind", [128, NPT * B_CAP], BF16, kind="ExternalInput"),
        nc.dram_tensor("img", [BI, T_IMG, D], F32, kind="ExternalInput"),
        nc.dram_tensor("wg1b", [128, NDT * H], BF16, kind="ExternalInput"),
        nc.dram_tensor("wg2b", [H, D], BF16, kind="ExternalInput"),
        nc.dram_tensor("wb1b", [128, NDT * H], BF16, kind="ExternalInput"),
        nc.dram_tensor("wb2b", [H, D], BF16, kind="ExternalInput"),
        nc.dram_tensor("bg1", [H, 1], F32, kind="ExternalInput"),
        nc.dram_tensor("bb1", [H, 1], F32, kind="ExternalInput"),
        nc.dram_tensor("bg2p1", [128, NDT], F32, kind="ExternalInput"),
        nc.dram_tensor("bb2t", [128, NDT], F32, kind="ExternalInput"),
        nc.dram_tensor("o36", [T_IMG, 1], F32, kind="ExternalInput"),
        nc.dram_tensor("out", [BI, B_CAP], F32, kind="ExternalOutput"),
    )


def _emit(nc, tc, capt_d, capr_d, ind_d, img_d, wg1_d, wg2_d, wb1_d, wb2_d,
          bg1_d, bb1_d, bg2p1_d, bb2_d, o36_d, out_d, reps_main=1):
    import contextlib
    ctx = contextlib.ExitStack()
    _xj = int(os.environ.get("KXJ", "10"))        # chunks via XBAR
    _cpact = os.environ.get("KCPACT", "0") == "1"  # alternate copies on ACT
    with ctx:
        const = ctx.enter_context(tc.tile_pool(name="const", bufs=2))
        capx = ctx.enter_context(tc.tile_pool(name="capx", bufs=1))
        imgs = ctx.enter_context(tc.tile_pool(name="imgs", bufs=2))
        work = ctx.enter_context(tc.tile_pool(name="work", bufs=3))
        small = ctx.enter_context(tc.tile_pool(name="small", bufs=1))
        actx = ctx.enter_context(contextlib.ExitStack())
        ppool = actx.enter_context(tc.tile_pool(name="psum", bufs=1,
                                                space="PSUM"))
        pacc = actx.enter_context(tc.tile_pool(name="pacc", bufs=1,
                                               space="PSUM"))

        ident = const.tile([128, 128], F32, bufs=1)
        masks.make_identity(nc, ident[:])
        ident_b = const.tile([128, 128], BF16, tag="identb", bufs=1)
        masks.make_identity(nc, ident_b[:])
        ones_b = const.tile([128, 1], BF16, tag="onesb", bufs=1)
        nc.gpsimd.memset(ones_b[:], 1.0)

        # ---- loads: smalls/weights/img first, then capR (stats), capT ----
        bg1_s = const.tile([H, 1], F32, tag="bg1", bufs=1)
        nc.sync.dma_start(out=bg1_s[:], in_=bg1_d[:])
        bb1_s = const.tile([H, 1], F32, tag="bb1", bufs=1)
        nc.sync.dma_start(out=bb1_s[:], in_=bb1_d[:])
        bg2p1_s = const.tile([128, NDT], F32, tag="bg2p1", bufs=1)
        nc.sync.dma_start(out=bg2p1_s[:], in_=bg2p1_d[:])
        bb2_s = const.tile([128, NDT], F32, tag="bb2t", bufs=1)
        nc.sync.dma_start(out=bb2_s[:], in_=bb2_d[:])
        o36_s = const.tile([T_IMG, 1], F32, tag="o36", bufs=1)
        nc.sync.dma_start(out=o36_s[:], in_=o36_d[:])
        wg1_b = const.tile([128, NDT * H], BF16, tag="wg1b", bufs=1)
        nc.sync.dma_start(out=wg1_b[:], in_=wg1_d[:])
        wb1_b = const.tile([128, NDT * H], BF16, tag="wb1b", bufs=1)
        nc.sync.dma_start(out=wb1_b[:], in_=wb1_d[:])
        wg2_b = const.tile([128, D], BF16, tag="wg2b", bufs=1)
        nc.sync.dma_start(out=wg2_b[:], in_=wg2_d[:])
        wb2_b = const.tile([128, D], BF16, tag="wb2b", bufs=1)
        nc.sync.dma_start(out=wb2_b[:], in_=wb2_d[:])
        img_tiles = []
        for i in range(BI):
            ichunk = imgs.tile([T_IMG, D], F32, tag="ichunk", bufs=4)
            nc.sync.dma_start(out=ichunk[:], in_=img_d[i])
            img_tiles.append(ichunk)

        ind_s = capx.tile([128, NPT * B_CAP], BF16, tag="ind", bufs=2)
        ind3 = ind_s[:].rearrange("p (c b) -> p c b", c=NPT)
        nc.sync.dma_start(out=ind_s[:], in_=ind_d[:])
        capR = capx.tile([128, NPT * D], BF16, tag="capR", bufs=2)
        capR3 = capR[:].rearrange("p (c d) -> p c d", c=NPT)
        capr_v = capr_d[:].rearrange("(c p) d -> p c d", p=128)
        for c in range(NPT):
            nc.sync.dma_start(out=capR3[:, c, :], in_=capr_v[:, c, :])
        capT = capx.tile([128, NDT * R], BF16, tag="capT", bufs=1)
        capT3 = capT[:].rearrange("p (c r) -> p c r", c=NDT)
        capt_v = capt_d[:].rearrange("(c p) r -> p c r", p=128)
        for dt in range(NDT):
            nc.sync.dma_start(out=capT3[:, dt, :], in_=capt_v[:, dt, :])

        # ---- BN stats on PE off capR chunks (chunk-major) ----
        mus_ps = pacc.tile([128, NDT], F32, tag="mus_ps")
        for c in range(NPT):
            for dt in range(NDT):
                nc.tensor.matmul(mus_ps[:, dt:dt + 1],
                                 lhsT=capR3[:, c, dt * 128:(dt + 1) * 128],
                                 rhs=ones_b[:],
                                 start=(c == 0), stop=(c == NPT - 1))
        sqsum = small.tile([128, NDT], F32, tag="sqsum", bufs=2)
        sq_tiles = []
        for k in range(NDT):
            sq_t = ppool.tile([128, 128], F32, tag=f"sq_ps{k % 3}")
            sq_tiles.append(sq_t)
        for wave, dts in enumerate((range(0, 3), range(3, 6), range(6, 8))):
            for c in range(NPT):
                for dt in dts:
                    nc.tensor.matmul(
                        sq_tiles[dt][:],
                        lhsT=capR3[:, c, dt * 128:(dt + 1) * 128],
                        rhs=capR3[:, c, dt * 128:(dt + 1) * 128],
                        start=(c == 0), stop=(c == NPT - 1))
            for dt in dts:
                dg = work.tile([128, 128], F32, tag="dg", bufs=2)
                nc.vector.tensor_tensor(dg[:], sq_tiles[dt][:], ident[:],
                                        op=ALU.mult)
                nc.vector.tensor_reduce(
                    sqsum[:, dt:dt + 1],
                    dg[:].rearrange("p (u q) -> p u q", u=1),
                    axis=AX.X, op=ALU.add)

        # mu = musum/R ; var = E[x^2]-mu^2 ; rs = Exp(-0.5*Ln(var+eps))
        mu = small.tile([128, NDT], F32, tag="mu", bufs=2)
        rs = small.tile([128, NDT], F32, tag="rs", bufs=2)
        tv = small.tile([128, NDT], F32, tag="tv", bufs=2)
        nc.vector.tensor_scalar_mul(mu[:], mus_ps[:], 1.0 / R)
        nc.vector.tensor_tensor(tv[:], mu[:], mu[:], op=ALU.mult)
        nc.vector.tensor_scalar(sqsum[:], sqsum[:], 1.0 / R, None,
                                op0=ALU.mult)
        nc.vector.tensor_tensor(tv[:], sqsum[:], tv[:], op=ALU.subtract)
        nc.vector.tensor_scalar_add(tv[:], tv[:], EPS)
        nc.scalar.activation(tv[:], tv[:], AF.Ln)
        nc.scalar.activation(rs[:], tv[:], AF.Exp, scale=-0.5)

        # ---- image means, directly transposed: imgrT [128, (dt i)] ----
        imgrT_ps = pacc.tile([128, NDT * BI], F32, tag="imgrT_ps")
        for i in range(BI):
            ichunk = img_tiles[i]
            for dt in range(NDT):
                nc.tensor.matmul(
                    imgrT_ps[:, dt * BI + i:dt * BI + i + 1],
                    lhsT=ichunk[:, dt * 128:(dt + 1) * 128], rhs=o36_s[:],
                    start=True, stop=True)

        imgrT = const.tile([128, NDT * BI], F32, tag="imgrT")
        imgrTb = const.tile([128, NDT * BI], BF16, tag="imgrTb")
        imgrT3 = imgrT[:].rearrange("p (c i) -> p c i", c=NDT)
        imgrTb3 = imgrTb[:].rearrange("p (c i) -> p c i", c=NDT)
        nc.vector.tensor_copy(imgrT[:], imgrT_ps[:])
        nc.scalar.copy(imgrTb[:], imgrT_ps[:])

        # 1/||v_i|| via accumulating [1,1] matmuls, rsqrt via Ln/Exp
        nrm2_ps = pacc.tile([1, BI], F32, tag="nrm2_ps")
        for i in range(BI):
            for dt in range(NDT):
                nc.tensor.matmul(
                    nrm2_ps[:, i:i + 1],
                    lhsT=imgrT3[:, dt, i:i + 1], rhs=imgrT3[:, dt, i:i + 1],
                    start=(dt == 0), stop=(dt == NDT - 1))
        rsr_row = small.tile([1, BI], F32, tag="rsr_row", bufs=2)
        nc.scalar.activation(rsr_row[:], nrm2_ps[:], AF.Ln)
        nc.scalar.activation(rsr_row[:], rsr_row[:], AF.Exp, scale=-0.5)

        # ---- CBN MLPs -> gamT/betT [128, (dt, i)] f32 ----
        wg1_b3 = wg1_b[:].rearrange("p (c h) -> p c h", c=NDT)
        wb1_b3 = wb1_b[:].rearrange("p (c h) -> p c h", c=NDT)

        def mlp_head(w1_b3, b1_s, w2_b, b2_s, name):
            h_ps = ppool.tile([H, BI], F32, tag="h_ps")
            for dt in range(NDT):
                nc.tensor.matmul(h_ps[:], lhsT=w1_b3[:, dt, :],
                                 rhs=imgrTb3[:, dt, :],
                                 start=(dt == 0), stop=(dt == NDT - 1))
            hT = small.tile([H, BI], BF16, tag=f"hT_{name}", bufs=2)
            nc.scalar.activation(hT[:], h_ps[:], AF.Relu, bias=b1_s[:],
                                 scale=1.0)
            outT = const.tile([128, NDT * BI], F32, tag=f"outT_{name}")
            outT3 = outT[:].rearrange("p (c i) -> p c i", c=NDT)
            for dt in range(NDT):
                o_ps = ppool.tile([128, BI], F32, tag="o_ps")
                nc.tensor.matmul(o_ps[:],
                                 lhsT=w2_b[:, dt * 128:(dt + 1) * 128],
                                 rhs=hT[:], start=True, stop=True)
                nc.scalar.activation(outT3[:, dt, :], o_ps[:], AF.Identity,
                                     bias=b2_s[:, dt:dt + 1], scale=1.0)
            return outT3

        gamT3 = mlp_head(wg1_b3, bg1_s, wg2_b, bg2p1_s, "g")
        betT3 = mlp_head(wb1_b3, bb1_s, wb2_b, bb2_s, "b")

        # ---- es = 10*gam*rs ; wb' = 10*bet - es*mu ----
        es = const.tile([128, NDT * BI], F32, tag="es")
        wb = const.tile([128, NDT * BI], F32, tag="wb")
        es3 = es[:].rearrange("p (c i) -> p c i", c=NDT)
        wb3 = wb[:].rearrange("p (c i) -> p c i", c=NDT)
        rs_b = rs[:].rearrange("p (c u) -> p c u", u=1).broadcast_to(
            [128, NDT, BI])
        mu_b = mu[:].rearrange("p (c u) -> p c u", u=1).broadcast_to(
            [128, NDT, BI])
        tmp64 = small.tile([128, NDT * BI], F32, tag="tmp64", bufs=2)
        tmp3 = tmp64[:].rearrange("p (c i) -> p c i", c=NDT)
        nc.vector.tensor_tensor(es3, gamT3, rs_b, op=ALU.mult)
        nc.vector.tensor_scalar_mul(es[:], es[:], 10.0)
        nc.vector.tensor_tensor(tmp3, es3, mu_b, op=ALU.mult)
        nc.vector.tensor_scalar_mul(wb[:], betT3.rearrange("p c i -> p (c i)"),
                                    10.0)
        nc.vector.tensor_tensor(wb[:], wb[:], tmp64[:], op=ALU.subtract)

        # ---- main loop ----
        actx.close()  # release phase psum banks
        pmain = ctx.enter_context(tc.tile_pool(name="pmain", bufs=1,
                                               space="PSUM"))
        psacc = ctx.enter_context(tc.tile_pool(name="psacc", bufs=2,
                                               space="PSUM"))
        ptr = None
        if _xj < NPT:
            ptr = ctx.enter_context(tc.tile_pool(name="ptr", bufs=2,
                                                 space="PSUM"))
        dot_ps = pmain.tile([1, BI * B_CAP], F32, tag="dot_ps")
        nrm_ps = pmain.tile([1, BI * B_CAP], F32, tag="nrm_ps")

        for _rep in range(reps_main):
            for i in range(BI):
                s12_ps = psacc.tile([128, 2 * NDT * B_CAP], F32, tag="s12")
                s12v = s12_ps[:].rearrange("p (s c b) -> p s c b", s=2, c=NDT)
                for dt in range(NDT):
                    e_t = work.tile([128, R], BF16, tag="e")
                    nc.scalar.activation(e_t[:], capT3[:, dt, :], AF.Exp,
                                         bias=0.0, scale=es3[:, dt, i:i + 1])
                    eT = work.tile([128, NPT * 128], BF16, tag="eT")
                    eT3 = eT[:].rearrange("p (c j) -> p c j", c=NPT)
                    if _xj > 0:
                        nc.sync.dma_start_transpose(eT3[:, 0:_xj, :],
                                                    e_t[:, 0:_xj * 128])
                    if _xj < NPT:
                        npe = NPT - _xj
                        done = 0
                        wv = 0
                        while done < npe:
                            n_w = min(8, npe - done)
                            tr_ps = ptr.tile([128, n_w * 128], BF16,
                                             tag="tr")
                            for k in range(n_w):
                                src = _xj + done + k
                                nc.tensor.transpose(
                                    tr_ps[:, k * 128:(k + 1) * 128],
                                    e_t[:, src * 128:(src + 1) * 128],
                                    ident_b[:])
                            dst = eT[:, (_xj + done) * 128:
                                     (_xj + done + n_w) * 128]
                            if _cpact and wv % 2 == 1:
                                nc.scalar.copy(dst, tr_ps[:])
                            else:
                                nc.vector.tensor_copy(dst, tr_ps[:])
                            done += n_w
                            wv += 1
                    qT = work.tile([128, NPT * 128], BF16, tag="qT")
                    qT3 = qT[:].rearrange("p (c j) -> p c j", c=NPT)
                    nc.vector.tensor_tensor(
                        qT3, eT3, capR3[:, :, dt * 128:(dt + 1) * 128],
                        op=ALU.mult)
                    for c in range(NPT):
                        nc.tensor.matmul(s12v[:, 0, dt, :], lhsT=eT3[:, c, :],
                                         rhs=ind3[:, c, :],
                                         start=(c == 0), stop=(c == NPT - 1))
                    for c in range(NPT):
                        nc.tensor.matmul(s12v[:, 1, dt, :], lhsT=qT3[:, c, :],
                                         rhs=ind3[:, c, :],
                                         start=(c == 0), stop=(c == NPT - 1))
                # sc = s2/s1 ; w' = es*sc + wb' ; dots on PE
                r1 = work.tile([128, NDT * B_CAP], F32, tag="r1", bufs=2)
                nc.vector.reciprocal(r1[:], s12_ps[:, 0:NDT * B_CAP])
                sc = work.tile([128, NDT * B_CAP], F32, tag="sc", bufs=2)
                nc.vector.tensor_tensor(
                    sc[:], s12_ps[:, NDT * B_CAP:2 * NDT * B_CAP], r1[:],
                    op=ALU.mult)
                for dt in range(NDT):
                    w_t = work.tile([128, B_CAP], BF16, tag="w")
                    nc.vector.tensor_scalar(
                        w_t[:], sc[:, dt * B_CAP:(dt + 1) * B_CAP],
                        es3[:, dt, i:i + 1], wb3[:, dt, i:i + 1],
                        op0=ALU.mult, op1=ALU.add)
                    w2_t = work.tile([128, B_CAP], BF16, tag="w2")
                    nc.gpsimd.tensor_tensor(w2_t[:], w_t[:], w_t[:],
                                            op=ALU.mult)
                    nc.tensor.matmul(dot_ps[:, i * B_CAP:(i + 1) * B_CAP],
                                     lhsT=imgrTb3[:, dt, i:i + 1], rhs=w_t[:],
                                     start=(dt == 0), stop=(dt == NDT - 1))
                    nc.tensor.matmul(nrm_ps[:, i * B_CAP:(i + 1) * B_CAP],
                                     lhsT=ones_b[:], rhs=w2_t[:],
                                     start=(dt == 0), stop=(dt == NDT - 1))

            # ---- epilogue: sims = dot * Exp(-0.5*Ln(nrm)) * (1/|v|) ----
            rsn = small.tile([1, BI * B_CAP], F32, tag="rsn")
            nc.scalar.activation(rsn[:], nrm_ps[:], AF.Ln)
            nc.scalar.activation(rsn[:], rsn[:], AF.Exp, scale=-0.5)
            prod = small.tile([1, BI * B_CAP], F32, tag="prod")
            nc.vector.tensor_tensor(prod[:], dot_ps[:], rsn[:], op=ALU.mult)
            res = small.tile([1, BI * B_CAP], F32, tag="res")
            rsr_b = rsr_row[:].rearrange("p (i u) -> p i u", u=1).broadcast_to(
                [1, BI, B_CAP])
            nc.vector.tensor_tensor(
                res[:].rearrange("p (i b) -> p i b", i=BI),
                prod[:].rearrange("p (i b) -> p i b", i=BI),
                rsr_b, op=ALU.mult)
            nc.sync.dma_start(out=out_d[:].rearrange("i b -> (i b)"),
                              in_=res[:])


def _build():
    nc = bacc.Bacc("TRN2", target_bir_lowering=False, debug=False,
                   num_devices=N_CORES)
    tensors = _declare_io(nc)
    with TileContext(nc) as tc:
        _emit(nc, tc, *tensors)
    nc.compile()
    return nc


def _build_repeated(reps):
    """Timing variant: run the compute `reps` times in one NEFF. With
    KREPMODE=main, phase A runs once and only the main loop repeats."""
    nc = bacc.Bacc("TRN2", target_bir_lowering=False, debug=False,
                   num_devices=N_CORES)
    tensors = _declare_io(nc)
    with TileContext(nc) as tc:
        if os.environ.get("KREPMODE") == "main":
            _emit(nc, tc, *tensors, reps_main=reps)
        else:
            for _ in range(reps):
                _emit(nc, tc, *tensors)
    nc.compile()
    return nc


def _get_compiled():
    global _COMPILED
    if _COMPILED is None:
        _COMPILED = _build()
    return _COMPILED


def _indicator():
    ind = np.zeros((128, NPT, B_CAP), np.float32)
    for c in range(NPT):
        for r in range(128):
            ind[r, c, (c * 128 + r) // T_IMG] = 1.0
    return ind.reshape(128, NPT * B_CAP)


def _in_maps(img_embed, cap_embed, Wg1, bg1, Wg2, bg2, Wb1, bb1, Wb2, bb2):
    cap = np.ascontiguousarray(
        cap_embed[:, :T_IMG, :].reshape(R, D)).astype(np.float32)

    def w1_tiles(W):
        return np.ascontiguousarray(
            W.reshape(NDT, 128, H).transpose(1, 0, 2).reshape(128, NDT * H)
        ).astype(BF16_NP)

    shared = {
        "capt": np.ascontiguousarray(cap.T).astype(BF16_NP),
        "capr": cap.astype(BF16_NP),
        "ind": _indicator().astype(BF16_NP),
        "wg1b": w1_tiles(np.asarray(Wg1, np.float32)),
        "wb1b": w1_tiles(np.asarray(Wb1, np.float32)),
        "wg2b": np.ascontiguousarray(Wg2, np.float32).astype(BF16_NP),
        "wb2b": np.ascontiguousarray(Wb2, np.float32).astype(BF16_NP),
        "bg1": np.ascontiguousarray(bg1.reshape(H, 1), np.float32),
        "bb1": np.ascontiguousarray(bb1.reshape(H, 1), np.float32),
        "bg2p1": np.ascontiguousarray((bg2 + 1.0).reshape(NDT, 128).T,
                                      np.float32),
        "bb2t": np.ascontiguousarray(bb2.reshape(NDT, 128).T, np.float32),
        "o36": np.full((T_IMG, 1), 1.0 / T_IMG, np.float32),
    }
    maps = []
    for c in range(N_CORES):
        m = dict(shared)
        m["img"] = np.ascontiguousarray(
            img_embed[c * BI:(c + 1) * BI], np.float32)
        maps.append(m)
    return maps


def kernel(img_embed, cap_embed, lens, Wg1, bg1, Wg2, bg2, Wb1, bb1, Wb2, bb2):
    del lens  # unused by the reference computation
    nc = _get_compiled()
    maps = _in_maps(np.asarray(img_embed), np.asarray(cap_embed),
                    np.asarray(Wg1), np.asarray(bg1), np.asarray(Wg2),
                    np.asarray(bg2), np.asarray(Wb1), np.asarray(bb1),
                    np.asarray(Wb2), np.asarray(bb2))
    import time as _time
    last = None
    for attempt in range(5):  # device occasionally needs runs to recover
        try:
            res = run_bass_kernel_spmd(nc, maps, core_ids=list(range(N_CORES)))
            break
        except Exception as e:
            last = e
            _time.sleep(10)
    else:
        raise last
    return np.concatenate([res.results[c]["out"] for c in range(N_CORES)],
                          axis=0).astype(np.float32)


# revision 38
# speedup vs baseline: 1.6935x; 1.1132x over previous
"""Trainium2 Bass kernel for nn_AdaptiveEmbeddingI2T.

Computes, for image-batch shard i on each of 8 NeuronCores:
  sims[i, b] = <img_vec_i, txt_vec_ib> with
  txt_vec_ib = l2norm_d( mean_t( softmax_t(10*(gam_id*xn_bdt+bet_id)) * (gam*xn+bet) ) )

Device-side algebra (per image i, channel d, caption b, time t):
  - softmax over t is shift/scale invariant in the ratio
      sc[d,b] = sum_t(e*cap) / sum_t(e),  e = exp(es[d,i]*cap[d,b,t]),
      es = 10*gam*rs   (the -es*mu shift and exp(bias) factor cancel)
  - txt_vec ~ w' = es*sc + (10*bet - es*mu)   (any uniform scale of w'
      cancels in the final l2 normalization, so the /36 and /10 drop)
  - sims = (sum_d v*w') * rsqrt(sum_d w'^2) * rsqrt(sum_d v^2)

Engine mapping:
  - exp on ACT in [d-partition, (b t)] layout (per-partition scale port)
  - e -> eT: one DMA-XBAR transpose for the first KXJ row-chunks, PE
    transposes + DVE psum->sbuf copy for the rest (balances DMA vs PE/DVE)
  - qT = eT * capR elementwise on DVE (the big DVE op)
  - s1 = sum_t e and s2 = sum_t q as PE indicator matmuls over the
    r=(b,t) partition chunks, accumulating f32 in PSUM (no DVE trees)
  - w' affine on DVE (tensor_scalar), w'^2 on GPSIMD, dots on PE
  - BN stats via PE matmuls (ones / self) off capR, diag extract on DVE
  - all rsqrt via Exp(-0.5*Ln(x)) so every ACT func lives in the single
    natural_log_exp_and_others table (no act-table reloads)

Sharding: image batch axis across 8 cores (8 images/core); cap + params
replicated; host concatenates the (8, 64) row blocks.
"""

import os
import sys

if "/opt/trn_rl_repo" not in sys.path:
    sys.path.insert(0, "/opt/trn_rl_repo")

import numpy as np
import ml_dtypes

import concourse.bacc as bacc
import concourse.mybir as mybir
from concourse import masks
from concourse.bass_utils import run_bass_kernel_spmd
from concourse.tile import TileContext

B_IMG, B_CAP, T_CAP, D = 64, 64, 64, 1024
H = 128
T_IMG = 36
EPS = 1e-5
N_CORES = 8
BI = B_IMG // N_CORES          # images per core
R = B_CAP * T_IMG              # 2304 caption rows
NPT = R // 128                 # 18 caption row chunks
NDT = D // 128                 # 8 channel tiles

F32 = mybir.dt.float32
BF16 = mybir.dt.bfloat16
BF16_NP = ml_dtypes.bfloat16

AF = mybir.ActivationFunctionType
ALU = mybir.AluOpType
AX = mybir.AxisListType

_COMPILED = None


def _declare_io(nc):
    return (
        nc.dram_tensor("capt", [D, R], BF16, kind="ExternalInput"),
        nc.dram_tensor("capr", [R, D], BF16, kind="ExternalInput"),
        nc.dram_tensor("ind", [128, NPT * B_CAP], BF16, kind="ExternalInput"),
        nc.dram_tensor("img", [BI, T_IMG, D], F32, kind="ExternalInput"),
        nc.dram_tensor("wg1b", [128, NDT * H], BF16, kind="ExternalInput"),
        nc.dram_tensor("wg2b", [H, D], BF16, kind="ExternalInput"),
        nc.dram_tensor("wb1b", [128, NDT * H], BF16, kind="ExternalInput"),
        nc.dram_tensor("wb2b", [H, D], BF16, kind="ExternalInput"),
        nc.dram_tensor("bg1", [H, 1], F32, kind="ExternalInput"),
        nc.dram_tensor("bb1", [H, 1], F32, kind="ExternalInput"),
        nc.dram_tensor("bg2p1", [128, NDT], F32, kind="ExternalInput"),
        nc.dram_tensor("bb2t", [128, NDT], F32, kind="ExternalInput"),
        nc.dram_tensor("o36", [T_IMG, 1], F32, kind="ExternalInput"),
        nc.dram_tensor("out", [BI, B_CAP], F32, kind="ExternalOutput"),
    )


def _emit(nc, tc, capt_d, capr_d, ind_d, img_d, wg1_d, wg2_d, wb1_d, wb2_d,
          bg1_d, bb1_d, bg2p1_d, bb2_d, o36_d, out_d, reps_main=1):
    import contextlib
    ctx = contextlib.ExitStack()
    _xj = int(os.environ.get("KXJ", "10"))        # chunks via XBAR
    _cpact = os.environ.get("KCPACT", "0") == "1"  # alternate copies on ACT
    with ctx:
        const = ctx.enter_context(tc.tile_pool(name="const", bufs=2))
        capx = ctx.enter_context(tc.tile_pool(name="capx", bufs=1))
        imgs = ctx.enter_context(tc.tile_pool(name="imgs", bufs=2))
        work = ctx.enter_context(tc.tile_pool(name="work", bufs=3))
        small = ctx.enter_context(tc.tile_pool(name="small", bufs=1))
        actx = ctx.enter_context(contextlib.ExitStack())
        ppool = actx.enter_context(tc.tile_pool(name="psum", bufs=1,
                                                space="PSUM"))
        pacc = actx.enter_context(tc.tile_pool(name="pacc", bufs=1,
                                               space="PSUM"))

        ident = const.tile([128, 128], F32, bufs=1)
        masks.make_identity(nc, ident[:])
        ident_b = const.tile([128, 128], BF16, tag="identb", bufs=1)
        masks.make_identity(nc, ident_b[:])
        ones_b = const.tile([128, 1], BF16, tag="onesb", bufs=1)
        nc.gpsimd.memset(ones_b[:], 1.0)

        # ---- loads: smalls/weights/img first, then capR (stats), capT ----
        bg1_s = const.tile([H, 1], F32, tag="bg1", bufs=1)
        nc.sync.dma_start(out=bg1_s[:], in_=bg1_d[:])
        bb1_s = const.tile([H, 1], F32, tag="bb1", bufs=1)
        nc.sync.dma_start(out=bb1_s[:], in_=bb1_d[:])
        bg2p1_s = const.tile([128, NDT], F32, tag="bg2p1", bufs=1)
        nc.sync.dma_start(out=bg2p1_s[:], in_=bg2p1_d[:])
        bb2_s = const.tile([128, NDT], F32, tag="bb2t", bufs=1)
        nc.sync.dma_start(out=bb2_s[:], in_=bb2_d[:])
        o36_s = const.tile([T_IMG, 1], F32, tag="o36", bufs=1)
        nc.sync.dma_start(out=o36_s[:], in_=o36_d[:])
        wg1_b = const.tile([128, NDT * H], BF16, tag="wg1b", bufs=1)
        nc.sync.dma_start(out=wg1_b[:], in_=wg1_d[:])
        wb1_b = const.tile([128, NDT * H], BF16, tag="wb1b", bufs=1)
        nc.sync.dma_start(out=wb1_b[:], in_=wb1_d[:])
        wg2_b = const.tile([128, D], BF16, tag="wg2b", bufs=1)
        nc.sync.dma_start(out=wg2_b[:], in_=wg2_d[:])
        wb2_b = const.tile([128, D], BF16, tag="wb2b", bufs=1)
        nc.sync.dma_start(out=wb2_b[:], in_=wb2_d[:])
        img_tiles = []
        for i in range(BI):
            ichunk = imgs.tile([T_IMG, D], F32, tag="ichunk", bufs=4)
            nc.sync.dma_start(out=ichunk[:], in_=img_d[i])
            img_tiles.append(ichunk)

        ind_s = capx.tile([128, NPT * B_CAP], BF16, tag="ind", bufs=2)
        ind3 = ind_s[:].rearrange("p (c b) -> p c b", c=NPT)
        nc.sync.dma_start(out=ind_s[:], in_=ind_d[:])
        capR = capx.tile([128, NPT * D], BF16, tag="capR", bufs=2)
        capR3 = capR[:].rearrange("p (c d) -> p c d", c=NPT)
        capr_v = capr_d[:].rearrange("(c p) d -> p c d", p=128)
        for c in range(NPT):
            nc.sync.dma_start(out=capR3[:, c, :], in_=capr_v[:, c, :])
        capT = capx.tile([128, NDT * R], BF16, tag="capT", bufs=1)
        capT3 = capT[:].rearrange("p (c r) -> p c r", c=NDT)
        capt_v = capt_d[:].rearrange("(c p) r -> p c r", p=128)
        for dt in range(NDT):
            nc.sync.dma_start(out=capT3[:, dt, :], in_=capt_v[:, dt, :])

        # ---- BN stats on PE off capR chunks (chunk-major) ----
        mus_ps = pacc.tile([128, NDT], F32, tag="mus_ps")
        for c in range(NPT):
            for dt in range(NDT):
                nc.tensor.matmul(mus_ps[:, dt:dt + 1],
                                 lhsT=capR3[:, c, dt * 128:(dt + 1) * 128],
                                 rhs=ones_b[:],
                                 start=(c == 0), stop=(c == NPT - 1))
        sqsum = small.tile([128, NDT], F32, tag="sqsum", bufs=2)
        sq_tiles = []
        for k in range(NDT):
            sq_t = ppool.tile([128, 128], F32, tag=f"sq_ps{k % 3}")
            sq_tiles.append(sq_t)
        for wave, dts in enumerate((range(0, 3), range(3, 6), range(6, 8))):
            for c in range(NPT):
                for dt in dts:
                    nc.tensor.matmul(
                        sq_tiles[dt][:],
                        lhsT=capR3[:, c, dt * 128:(dt + 1) * 128],
                        rhs=capR3[:, c, dt * 128:(dt + 1) * 128],
                        start=(c == 0), stop=(c == NPT - 1))
            for dt in dts:
                dg = work.tile([128, 128], F32, tag="dg", bufs=2)
                nc.vector.tensor_tensor(dg[:], sq_tiles[dt][:], ident[:],
                                        op=ALU.mult)
                nc.vector.tensor_reduce(
                    sqsum[:, dt:dt + 1],
                    dg[:].rearrange("p (u q) -> p u q", u=1),
                    axis=AX.X, op=ALU.add)

        # mu = musum/R ; var = E[x^2]-mu^2 ; rs = Exp(-0.5*Ln(var+eps))
        mu = small.tile([128, NDT], F32, tag="mu", bufs=2)
        rs = small.tile([128, NDT], F32, tag="rs", bufs=2)
        tv = small.tile([128, NDT], F32, tag="tv", bufs=2)
        nc.vector.tensor_scalar_mul(mu[:], mus_ps[:], 1.0 / R)
        nc.vector.tensor_tensor(tv[:], mu[:], mu[:], op=ALU.mult)
        nc.vector.tensor_scalar(sqsum[:], sqsum[:], 1.0 / R, None,
                                op0=ALU.mult)
        nc.vector.tensor_tensor(tv[:], sqsum[:], tv[:], op=ALU.subtract)
        nc.vector.tensor_scalar_add(tv[:], tv[:], EPS)
        nc.scalar.activation(tv[:], tv[:], AF.Ln)
        nc.scalar.activation(rs[:], tv[:], AF.Exp, scale=-0.5)

        # ---- image means, directly transposed: imgrT [128, (dt i)] ----
        imgrT_ps = pacc.tile([128, NDT * BI], F32, tag="imgrT_ps")
        for i in range(BI):
            ichunk = img_tiles[i]
            for dt in range(NDT):
                nc.tensor.matmul(
                    imgrT_ps[:, dt * BI + i:dt * BI + i + 1],
                    lhsT=ichunk[:, dt * 128:(dt + 1) * 128], rhs=o36_s[:],
                    start=True, stop=True)

        imgrT = const.tile([128, NDT * BI], F32, tag="imgrT")
        imgrTb = const.tile([128, NDT * BI], BF16, tag="imgrTb")
        imgrT3 = imgrT[:].rearrange("p (c i) -> p c i", c=NDT)
        imgrTb3 = imgrTb[:].rearrange("p (c i) -> p c i", c=NDT)
        nc.vector.tensor_copy(imgrT[:], imgrT_ps[:])
        nc.scalar.copy(imgrTb[:], imgrT_ps[:])

        # 1/||v_i|| via accumulating [1,1] matmuls, rsqrt via Ln/Exp
        nrm2_ps = pacc.tile([1, BI], F32, tag="nrm2_ps")
        for i in range(BI):
            for dt in range(NDT):
                nc.tensor.matmul(
                    nrm2_ps[:, i:i + 1],
                    lhsT=imgrT3[:, dt, i:i + 1], rhs=imgrT3[:, dt, i:i + 1],
                    start=(dt == 0), stop=(dt == NDT - 1))
        rsr_row = small.tile([1, BI], F32, tag="rsr_row", bufs=2)
        nc.scalar.activation(rsr_row[:], nrm2_ps[:], AF.Ln)
        nc.scalar.activation(rsr_row[:], rsr_row[:], AF.Exp, scale=-0.5)

        # ---- CBN MLPs -> gamT/betT [128, (dt, i)] f32 ----
        wg1_b3 = wg1_b[:].rearrange("p (c h) -> p c h", c=NDT)
        wb1_b3 = wb1_b[:].rearrange("p (c h) -> p c h", c=NDT)

        def mlp_head(w1_b3, b1_s, w2_b, b2_s, name):
            h_ps = ppool.tile([H, BI], F32, tag="h_ps")
            for dt in range(NDT):
                nc.tensor.matmul(h_ps[:], lhsT=w1_b3[:, dt, :],
                                 rhs=imgrTb3[:, dt, :],
                                 start=(dt == 0), stop=(dt == NDT - 1))
            hT = small.tile([H, BI], BF16, tag=f"hT_{name}", bufs=2)
            nc.scalar.activation(hT[:], h_ps[:], AF.Relu, bias=b1_s[:],
                                 scale=1.0)
            outT = const.tile([128, NDT * BI], F32, tag=f"outT_{name}")
            outT3 = outT[:].rearrange("p (c i) -> p c i", c=NDT)
            for dt in range(NDT):
                o_ps = ppool.tile([128, BI], F32, tag="o_ps")
                nc.tensor.matmul(o_ps[:],
                                 lhsT=w2_b[:, dt * 128:(dt + 1) * 128],
                                 rhs=hT[:], start=True, stop=True)
                nc.scalar.activation(outT3[:, dt, :], o_ps[:], AF.Identity,
                                     bias=b2_s[:, dt:dt + 1], scale=1.0)
            return outT3

        gamT3 = mlp_head(wg1_b3, bg1_s, wg2_b, bg2p1_s, "g")
        betT3 = mlp_head(wb1_b3, bb1_s, wb2_b, bb2_s, "b")

        # ---- es = 10*gam*rs ; wb' = 10*bet - es*mu ----
        es = const.tile([128, NDT * BI], F32, tag="es")
        wb = const.tile([128, NDT * BI], F32, tag="wb")
        es3 = es[:].rearrange("p (c i) -> p c i", c=NDT)
        wb3 = wb[:].rearrange("p (c i) -> p c i", c=NDT)
        rs_b = rs[:].rearrange("p (c u) -> p c u", u=1).broadcast_to(
            [128, NDT, BI])
        mu_b = mu[:].rearrange("p (c u) -> p c u", u=1).broadcast_to(
            [128, NDT, BI])
        tmp64 = small.tile([128, NDT * BI], F32, tag="tmp64", bufs=2)
        tmp3 = tmp64[:].rearrange("p (c i) -> p c i", c=NDT)
        nc.vector.tensor_tensor(es3, gamT3, rs_b, op=ALU.mult)
        nc.vector.tensor_scalar_mul(es[:], es[:], 10.0)
        nc.vector.tensor_tensor(tmp3, es3, mu_b, op=ALU.mult)
        nc.vector.tensor_scalar_mul(wb[:], betT3.rearrange("p c i -> p (c i)"),
                                    10.0)
        nc.vector.tensor_tensor(wb[:], wb[:], tmp64[:], op=ALU.subtract)

        # ---- main loop ----
        actx.close()  # release phase psum banks
        pmain = ctx.enter_context(tc.tile_pool(name="pmain", bufs=1,
                                               space="PSUM"))
        psacc = ctx.enter_context(tc.tile_pool(name="psacc", bufs=2,
                                               space="PSUM"))
        ptr = None
        if _xj < NPT:
            ptr = ctx.enter_context(tc.tile_pool(name="ptr", bufs=2,
                                                 space="PSUM"))
        dot_ps = pmain.tile([1, BI * B_CAP], F32, tag="dot_ps")
        nrm_ps = pmain.tile([1, BI * B_CAP], F32, tag="nrm_ps")

        for _rep in range(reps_main):
            for i in range(BI):
                s12_ps = psacc.tile([128, 2 * NDT * B_CAP], F32, tag="s12")
                s12v = s12_ps[:].rearrange("p (s c b) -> p s c b", s=2, c=NDT)
                for dt in range(NDT):
                    e_t = work.tile([128, R], BF16, tag="e", bufs=4)
                    nc.scalar.activation(e_t[:], capT3[:, dt, :], AF.Exp,
                                         bias=0.0, scale=es3[:, dt, i:i + 1])
                    eT = work.tile([128, NPT * 128], BF16, tag="eT")
                    eT3 = eT[:].rearrange("p (c j) -> p c j", c=NPT)
                    if _xj > 0:
                        nc.sync.dma_start_transpose(eT3[:, 0:_xj, :],
                                                    e_t[:, 0:_xj * 128])
                    if _xj < NPT:
                        npe = NPT - _xj
                        done = 0
                        wv = 0
                        while done < npe:
                            n_w = min(8, npe - done)
                            tr_ps = ptr.tile([128, n_w * 128], BF16,
                                             tag="tr")
                            for k in range(n_w):
                                src = _xj + done + k
                                nc.tensor.transpose(
                                    tr_ps[:, k * 128:(k + 1) * 128],
                                    e_t[:, src * 128:(src + 1) * 128],
                                    ident_b[:])
                            dst = eT[:, (_xj + done) * 128:
                                     (_xj + done + n_w) * 128]
                            if _cpact and wv % 2 == 1:
                                nc.scalar.copy(dst, tr_ps[:])
                            else:
                                nc.vector.tensor_copy(dst, tr_ps[:])
                            done += n_w
                            wv += 1
                    qT = work.tile([128, NPT * 128], BF16, tag="qT")
                    qT3 = qT[:].rearrange("p (c j) -> p c j", c=NPT)
                    nc.vector.tensor_tensor(
                        qT3, eT3, capR3[:, :, dt * 128:(dt + 1) * 128],
                        op=ALU.mult)
                    for c in range(NPT):
                        nc.tensor.matmul(s12v[:, 0, dt, :], lhsT=eT3[:, c, :],
                                         rhs=ind3[:, c, :],
                                         start=(c == 0), stop=(c == NPT - 1))
                    for c in range(NPT):
                        nc.tensor.matmul(s12v[:, 1, dt, :], lhsT=qT3[:, c, :],
                                         rhs=ind3[:, c, :],
                                         start=(c == 0), stop=(c == NPT - 1))
                # sc = s2/s1 ; w' = es*sc + wb' ; dots on PE
                r1 = work.tile([128, NDT * B_CAP], F32, tag="r1", bufs=2)
                nc.vector.reciprocal(r1[:], s12_ps[:, 0:NDT * B_CAP])
                sc = work.tile([128, NDT * B_CAP], F32, tag="sc", bufs=2)
                nc.vector.tensor_tensor(
                    sc[:], s12_ps[:, NDT * B_CAP:2 * NDT * B_CAP], r1[:],
                    op=ALU.mult)
                for dt in range(NDT):
                    w_t = work.tile([128, B_CAP], BF16, tag="w")
                    nc.vector.tensor_scalar(
                        w_t[:], sc[:, dt * B_CAP:(dt + 1) * B_CAP],
                        es3[:, dt, i:i + 1], wb3[:, dt, i:i + 1],
                        op0=ALU.mult, op1=ALU.add)
                    w2_t = work.tile([128, B_CAP], BF16, tag="w2")
                    nc.gpsimd.tensor_tensor(w2_t[:], w_t[:], w_t[:],
                                            op=ALU.mult)
                    nc.tensor.matmul(dot_ps[:, i * B_CAP:(i + 1) * B_CAP],
                                     lhsT=imgrTb3[:, dt, i:i + 1], rhs=w_t[:],
                                     start=(dt == 0), stop=(dt == NDT - 1))
                    nc.tensor.matmul(nrm_ps[:, i * B_CAP:(i + 1) * B_CAP],
                                     lhsT=ones_b[:], rhs=w2_t[:],
                                     start=(dt == 0), stop=(dt == NDT - 1))

            # ---- epilogue: sims = dot * Exp(-0.5*Ln(nrm)) * (1/|v|) ----
            rsn = small.tile([1, BI * B_CAP], F32, tag="rsn")
            nc.scalar.activation(rsn[:], nrm_ps[:], AF.Ln)
            nc.scalar.activation(rsn[:], rsn[:], AF.Exp, scale=-0.5)
            prod = small.tile([1, BI * B_CAP], F32, tag="prod")
            nc.vector.tensor_tensor(prod[:], dot_ps[:], rsn[:], op=ALU.mult)
            res = small.tile([1, BI * B_CAP], F32, tag="res")
            rsr_b = rsr_row[:].rearrange("p (i u) -> p i u", u=1).broadcast_to(
                [1, BI, B_CAP])
            nc.vector.tensor_tensor(
                res[:].rearrange("p (i b) -> p i b", i=BI),
                prod[:].rearrange("p (i b) -> p i b", i=BI),
                rsr_b, op=ALU.mult)
            nc.sync.dma_start(out=out_d[:].rearrange("i b -> (i b)"),
                              in_=res[:])


def _build():
    nc = bacc.Bacc("TRN2", target_bir_lowering=False, debug=False,
                   num_devices=N_CORES)
    tensors = _declare_io(nc)
    with TileContext(nc) as tc:
        _emit(nc, tc, *tensors)
    nc.compile()
    return nc


def _build_repeated(reps):
    """Timing variant: run the compute `reps` times in one NEFF. With
    KREPMODE=main, phase A runs once and only the main loop repeats."""
    nc = bacc.Bacc("TRN2", target_bir_lowering=False, debug=False,
                   num_devices=N_CORES)
    tensors = _declare_io(nc)
    with TileContext(nc) as tc:
        if os.environ.get("KREPMODE") == "main":
            _emit(nc, tc, *tensors, reps_main=reps)
        else:
            for _ in range(reps):
                _emit(nc, tc, *tensors)
    nc.compile()
    return nc


def _get_compiled():
    global _COMPILED
    if _COMPILED is None:
        _COMPILED = _build()
    return _COMPILED


def _indicator():
    ind = np.zeros((128, NPT, B_CAP), np.float32)
    for c in range(NPT):
        for r in range(128):
            ind[r, c, (c * 128 + r) // T_IMG] = 1.0
    return ind.reshape(128, NPT * B_CAP)


def _in_maps(img_embed, cap_embed, Wg1, bg1, Wg2, bg2, Wb1, bb1, Wb2, bb2):
    cap = np.ascontiguousarray(
        cap_embed[:, :T_IMG, :].reshape(R, D)).astype(np.float32)

    def w1_tiles(W):
        return np.ascontiguousarray(
            W.reshape(NDT, 128, H).transpose(1, 0, 2).reshape(128, NDT * H)
        ).astype(BF16_NP)

    shared = {
        "capt": np.ascontiguousarray(cap.T).astype(BF16_NP),
        "capr": cap.astype(BF16_NP),
        "ind": _indicator().astype(BF16_NP),
        "wg1b": w1_tiles(np.asarray(Wg1, np.float32)),
        "wb1b": w1_tiles(np.asarray(Wb1, np.float32)),
        "wg2b": np.ascontiguousarray(Wg2, np.float32).astype(BF16_NP),
        "wb2b": np.ascontiguousarray(Wb2, np.float32).astype(BF16_NP),
        "bg1": np.ascontiguousarray(bg1.reshape(H, 1), np.float32),
        "bb1": np.ascontiguousarray(bb1.reshape(H, 1), np.float32),
        "bg2p1": np.ascontiguousarray((bg2 + 1.0).reshape(NDT, 128).T,
                                      np.float32),
        "bb2t": np.ascontiguousarray(bb2.reshape(NDT, 128).T, np.float32),
        "o36": np.full((T_IMG, 1), 1.0 / T_IMG, np.float32),
    }
    maps = []
    for c in range(N_CORES):
        m = dict(shared)
        m["img"] = np.ascontiguousarray(
            img_embed[c * BI:(c + 1) * BI], np.float32)
        maps.append(m)
    return maps


def kernel(img_embed, cap_embed, lens, Wg1, bg1, Wg2, bg2, Wb1, bb1, Wb2, bb2):
    del lens  # unused by the reference computation
    nc = _get_compiled()
    maps = _in_maps(np.asarray(img_embed), np.asarray(cap_embed),
                    np.asarray(Wg1), np.asarray(bg1), np.asarray(Wg2),
                    np.asarray(bg2), np.asarray(Wb1), np.asarray(bb1),
                    np.asarray(Wb2), np.asarray(bb2))
    import time as _time
    last = None
    for attempt in range(5):  # device occasionally needs runs to recover
        try:
            res = run_bass_kernel_spmd(nc, maps, core_ids=list(range(N_CORES)))
            break
        except Exception as e:
            last = e
            _time.sleep(10)
    else:
        raise last
    return np.concatenate([res.results[c]["out"] for c in range(N_CORES)],
                          axis=0).astype(np.float32)
